# revision 12
# baseline (speedup 1.0000x reference)
"""Trainium2 Bass kernel for nn_DescriptorNetwork (gnn_message_passing).

Strategy:
 - Shard by crystal: core k owns crystals [3200k, 3200(k+1)) -> nodes/edges
   contiguous. Params replicated. All gathers are crystal-local.
 - Feature-major on-chip layout: feaT [64, nodes]; per-superblock streaming
   (640 crystals = 3200 nodes = 16000 edges per SB, 5 SBs per core).
 - Hidden layers of gate/msg nets: bf16 matmuls on catT [128, edges].
   cat gather = AP tricks (each crystal fully connected, 5 nodes).
 - gate output row rides the msg matmul as a 65th lhsT column -> one
   [65, chunk] PSUM tile, one ACT/DVE drain pass.
 - Attention softmax on [128, 125] tiles (partition p owns 5 crystals),
   reached via SBUF->SBUF reshape DMA of the gate row.
 - exp stabilization: constant per-head shift G (baked into drain bias),
   valid because softmax is shift-invariant; G chosen from an exact
   host-side forward of a sample of crystals so exp stays in range.
 - coeff broadcast across 64 feature partitions via PE outer product
   (ones[1,64] x coeff-row), PSUM-accumulation-free.
 - segment sums (5 edges/node, 5 nodes/crystal) = strided tensor_reduce.
"""
import sys
sys.path.insert(0, "/opt/trn_rl_repo")

import numpy as np
import ml_dtypes

import concourse.bass as bass
import concourse.bacc as bacc
import concourse.tile as tile
import concourse.mybir as mybir
from concourse.bass_utils import run_bass_kernel_spmd

F32 = mybir.dt.float32
F32R = mybir.dt.float32r
BF16 = mybir.dt.bfloat16
AF = mybir.ActivationFunctionType
OP = mybir.AluOpType
BF = ml_dtypes.bfloat16

NCORES = 8
C = 25600
NPC = 5
FEA = 64
EMB = 200
HID = 256
NG = 3          # graph layers
NH = 3          # heads per layer / cry heads
C_CORE = C // NCORES            # 3200 crystals
N_CORE = C_CORE * NPC           # 16000 nodes
E_CORE = N_CORE * NPC           # 80000 edges
SB_CR = 640                     # crystals per superblock
NSB = C_CORE // SB_CR           # 5
N_SB = SB_CR * NPC              # 3200 nodes / SB
E_SB = N_SB * NPC               # 16000 edges / SB
ECH = 500                       # edge chunk (<=512 psum cols)
NECH = E_SB // ECH              # 32
CRCH = 400                      # cry node chunk
NCRCH = N_SB // CRCH            # 8
EPP = E_SB // 128               # 125 edges per partition (5 crystals)
NPP = N_SB // 128               # 25 nodes per partition
CPP = SB_CR // 128              # 5 crystals per partition
NGRP = 640                      # emb node group
EMSG_W = 1000                   # emsg window (edges)
CEMSG_W = 800                   # cry emsg window (nodes)

_CACHE = {}


def _build():
    nc = bacc.Bacc(None, target_bir_lowering=False)

    ef = nc.dram_tensor("ef", [N_CORE, EMB], F32, kind="ExternalInput")
    ew = nc.dram_tensor("ew", [1, N_CORE], F32R, kind="ExternalInput")
    wfg = nc.dram_tensor("wfg", [NG * NH, NSB, 128, NPP], F32, kind="ExternalInput")
    wfc = nc.dram_tensor("wfc", [NH, NSB, 128, NPP], F32, kind="ExternalInput")
    w1 = nc.dram_tensor("w1", [128, 36 * 128], BF16, kind="ExternalInput")
    w2 = nc.dram_tensor("w2", [128, 36 * 65], F32, kind="ExternalInput")
    b1 = nc.dram_tensor("b1", [128, 36], F32, kind="ExternalInput")
    b2 = nc.dram_tensor("b2", [65, 12], F32, kind="ExternalInput")
    w1c = nc.dram_tensor("w1c", [64, 12 * 128], F32, kind="ExternalInput")
    w2c = nc.dram_tensor("w2c", [128, 12 * 65], F32, kind="ExternalInput")
    b1c = nc.dram_tensor("b1c", [128, 12], F32, kind="ExternalInput")
    wemba = nc.dram_tensor("wemba", [128, 63], F32, kind="ExternalInput")
    wembb = nc.dram_tensor("wembb", [72, 63], F32, kind="ExternalInput")
    bemb = nc.dram_tensor("bemb", [63, 1], F32, kind="ExternalInput")
    ident = nc.dram_tensor("ident", [128, 128], F32, kind="ExternalInput")
    out = nc.dram_tensor("out", [C_CORE, FEA], F32, kind="ExternalOutput")
    import os
    if os.environ.get("KERNEL_DEBUG") == "1":
        dbg_fea = nc.dram_tensor("dbg_fea", [64, N_SB], F32, kind="ExternalOutput")
        dbg_mg = nc.dram_tensor("dbg_mg", [65, E_SB], F32, kind="ExternalOutput")
        dbg_coeff = nc.dram_tensor("dbg_coeff", [128, EPP], F32, kind="ExternalOutput")
        dbg_fea1 = nc.dram_tensor("dbg_fea1", [64, N_SB], F32, kind="ExternalOutput")
    else:
        dbg_fea = dbg_mg = dbg_coeff = dbg_fea1 = None

    with tile.TileContext(nc) as tc:
        _emit(nc, tc, locals())
    nc.finalize()
    return nc


def _emit(nc, tc, t):
    ef, ew, wfg, wfc = t["ef"], t["ew"], t["wfg"], t["wfc"]
    w1, w2, b1, b2 = t["w1"], t["w2"], t["b1"], t["b2"]
    w1c, w2c, b1c = t["w1c"], t["w2c"], t["b1c"]
    wemba, wembb, bemb, ident, out = t["wemba"], t["wembb"], t["bemb"], t["ident"], t["out"]
    dbg_fea, dbg_mg, dbg_coeff, dbg_fea1 = t["dbg_fea"], t["dbg_mg"], t["dbg_coeff"], t["dbg_fea1"]

    import contextlib
    ctx = contextlib.ExitStack()
    with ctx:
        const = ctx.enter_context(tc.tile_pool(name="const", bufs=1))
        p_fea = ctx.enter_context(tc.tile_pool(name="p_fea", bufs=1))
        p_cat = ctx.enter_context(tc.tile_pool(name="p_cat", bufs=1))
        p_mg = ctx.enter_context(tc.tile_pool(name="p_mg", bufs=1))
        p_h1 = ctx.enter_context(tc.tile_pool(name="p_h1", bufs=2))
        p_sm = ctx.enter_context(tc.tile_pool(name="p_sm", bufs=2))
        p_c4 = ctx.enter_context(tc.tile_pool(name="p_c4", bufs=1))
        p_em = ctx.enter_context(tc.tile_pool(name="p_em", bufs=2))
        p_ef = ctx.enter_context(tc.tile_pool(name="p_ef", bufs=3))
        p_eft = ctx.enter_context(tc.tile_pool(name="p_eft", bufs=1))
        p_out = ctx.enter_context(tc.tile_pool(name="p_out", bufs=1))
        p_wf = ctx.enter_context(tc.tile_pool(name="p_wf", bufs=3))

        ps_h1 = ctx.enter_context(tc.tile_pool(name="ps_h1", bufs=1, space="PSUM"))
        ps_mg = ctx.enter_context(tc.tile_pool(name="ps_mg", bufs=2, space="PSUM"))
        ps_ms = ctx.enter_context(tc.tile_pool(name="ps_ms", bufs=2, space="PSUM"))

        # ---- resident params ----
        w1_t = const.tile([128, 36 * 128], BF16, name="w1t")
        nc.sync.dma_start(out=w1_t, in_=w1[:, :])
        b1_t = const.tile([128, 36], F32, name="b1t")
        nc.sync.dma_start(out=b1_t, in_=b1[:, :])
        b2_t = const.tile([65, 12], F32, name="b2t")
        nc.sync.dma_start(out=b2_t, in_=b2[:, :])
        b1c_t = const.tile([128, 12], F32, name="b1ct")
        nc.sync.dma_start(out=b1c_t, in_=b1c[:, :])
        bemb_t = const.tile([63, 1], F32, name="bembt")
        nc.sync.dma_start(out=bemb_t, in_=bemb[:, :])
        id_t = const.tile([128, 128], F32, name="idt")
        nc.sync.dma_start(out=id_t, in_=ident[:, :])

        # f32r params go through a DVE copy (producer must round to f32r)
        w2_t = const.tile([128, 36 * 65], F32R, name="w2t")
        w1c_t = const.tile([64, 12 * 128], F32R, name="w1ct")
        w2c_t = const.tile([128, 12 * 65], F32R, name="w2ct")
        wea_t = const.tile([128, 63], F32R, name="weat")
        web_t = const.tile([72, 63], F32R, name="webt")
        with tc.tile_pool(name="stg", bufs=1) as stg:
            for (dst, srcd, shp) in ((w2_t, w2, [128, 36 * 65]),
                                     (w1c_t, w1c, [64, 12 * 128]),
                                     (w2c_t, w2c, [128, 12 * 65]),
                                     (wea_t, wemba, [128, 63]),
                                     (web_t, wembb, [72, 63])):
                for c0 in range(0, shp[1], 640):
                    c1 = min(c0 + 640, shp[1])
                    s = stg.tile([128, 640], F32, name="stg_s", tag="s")
                    nc.sync.dma_start(out=s[0:shp[0], 0:c1 - c0],
                                      in_=srcd[:, c0:c1])
                    nc.vector.tensor_copy(out=dst[:, c0:c1],
                                          in_=s[0:shp[0], 0:c1 - c0])
        ones_s = const.tile([128, 64], F32, name="oness")
        nc.vector.memset(ones_s, 1.0)
        ones_t = const.tile([128, 64], F32R, name="onest")
        nc.vector.tensor_copy(out=ones_t, in_=ones_s)

        def bcast_inner5(ap2d, n):
            """[P, n] -> [P, n, 5] with 0-step inner (broadcast each col 5x)."""
            return bass.AP(tensor=ap2d.tensor, offset=ap2d.offset,
                           ap=[ap2d.ap[0], [ap2d.ap[1][0], n], [0, 5]])

        for sb in range(NSB):
            nbase = sb * N_SB

            # ================= embedding =================
            feaT = p_fea.tile([64, N_SB], F32R, name="feaT")
            for g in range(N_SB // NGRP):
                hi = p_eft.tile([128, NGRP], F32R, name="efhi")
                lo = p_eft.tile([72, NGRP], F32R, name="eflo")
                for tt in range(NGRP // 128):
                    rows = nbase + g * NGRP + tt * 128
                    eft = p_ef.tile([128, EMB], F32, name="eft")
                    nc.sync.dma_start(out=eft, in_=ef[rows:rows + 128, :])
                    ph = ps_ms.tile([128, 128], F32, name="pst", tag="ms")
                    nc.tensor.transpose(ph, eft[:, 0:128], id_t)
                    nc.vector.tensor_copy(out=hi[:, tt * 128:(tt + 1) * 128], in_=ph)
                    pl = ps_ms.tile([72, 128], F32, name="psl", tag="ms")
                    nc.tensor.transpose(pl, eft[:, 128:200], id_t)
                    nc.scalar.copy(out=lo[:, tt * 128:(tt + 1) * 128], in_=pl)
                for half in range(2):
                    cols = slice(half * (NGRP // 2), (half + 1) * (NGRP // 2))
                    pe = ps_ms.tile([63, NGRP // 2], F32, name="pse", tag="ms")
                    nc.tensor.matmul(out=pe, lhsT=wea_t, rhs=hi[:, cols],
                                     start=True, stop=False)
                    nc.tensor.matmul(out=pe, lhsT=web_t, rhs=lo[:, cols],
                                     start=False, stop=True)
                    dst = slice(g * NGRP + half * (NGRP // 2),
                                g * NGRP + (half + 1) * (NGRP // 2))
                    nc.scalar.activation(out=feaT[0:63, dst], in_=pe,
                                         func=AF.Identity, bias=bemb_t, scale=1.0)
            nc.sync.dma_start(out=feaT[63:64, :],
                              in_=ew[0:1, nbase:nbase + N_SB])
            if sb == 0 and dbg_fea is not None:
                nc.sync.dma_start(out=dbg_fea[:, :], in_=feaT.bitcast(F32))

            # ================= graph layers =================
            for l in range(NG):
                catT = p_cat.tile([128, E_SB], BF16, name="catT")
                for c in range(NECH):
                    crys = slice((c * ECH) // 25 * 5, ((c + 1) * ECH) // 25 * 5)
                    # self: node = 5cr + i, repeated 5x ; nbr: tile of 5
                    base = feaT[:, crys]
                    self_ap = bass.AP(tensor=base.tensor, offset=base.offset,
                                      ap=[base.ap[0], [5, ECH // 25], [1, 5], [0, 5]])
                    nbr_ap = bass.AP(tensor=base.tensor, offset=base.offset,
                                     ap=[base.ap[0], [5, ECH // 25], [0, 5], [1, 5]])
                    oc = catT[:, c * ECH:(c + 1) * ECH]
                    o3 = oc.rearrange("p (a b c) -> p a b c", b=5, c=5)
                    nc.vector.tensor_copy(out=o3[0:64], in_=self_ap)
                    nc.vector.tensor_copy(out=o3[64:128], in_=nbr_ap)

                for h in range(NH):
                    hl = l * NH + h
                    mgT = p_mg.tile([65, E_SB], F32, name="mgT", tag="mgT")
                    for c in range(NECH):
                        rsl = slice(c * ECH, (c + 1) * ECH)
                        hg = ps_h1.tile([128, 1024], F32, name="hg")
                        hm = ps_h1.tile([128, 1024], F32, name="hm")
                        for blk in range(2):
                            psl = slice(blk * 512, blk * 512 + ECH)
                            wcol = ((hl * 2 + 0) * 2 + blk) * 128
                            nc.tensor.matmul(out=hg[:, psl],
                                             lhsT=w1_t[:, wcol:wcol + 128],
                                             rhs=catT[:, rsl], start=True, stop=True)
                            wcol = ((hl * 2 + 1) * 2 + blk) * 128
                            nc.tensor.matmul(out=hm[:, psl],
                                             lhsT=w1_t[:, wcol:wcol + 128],
                                             rhs=catT[:, rsl], start=True, stop=True)
                        hgs = p_h1.tile([128, 2 * ECH], F32R, name="hgs")
                        hms = p_h1.tile([128, 2 * ECH], F32R, name="hms")
                        for blk in range(2):
                            bsl = slice(blk * ECH, (blk + 1) * ECH)
                            psl = slice(blk * 512, blk * 512 + ECH)
                            bg = (hl * 2 + 0) * 2 + blk
                            bm = (hl * 2 + 1) * 2 + blk
                            nc.scalar.activation(out=hgs[:, bsl], in_=hg[:, psl],
                                                 func=AF.Lrelu,
                                                 bias=b1_t[:, bg:bg + 1],
                                                 scale=1.0, alpha=0.01)
                            nc.scalar.activation(out=hms[:, bsl], in_=hm[:, psl],
                                                 func=AF.Lrelu,
                                                 bias=b1_t[:, bm:bm + 1],
                                                 scale=1.0, alpha=0.01)
                        pm = ps_mg.tile([65, ECH], F32, name="pm")
                        wb = hl * 4 * 65
                        nc.tensor.matmul(out=pm, lhsT=w2_t[:, wb:wb + 65],
                                         rhs=hms[:, 0:ECH], start=True, stop=False)
                        nc.tensor.matmul(out=pm, lhsT=w2_t[:, wb + 65:wb + 130],
                                         rhs=hms[:, ECH:2 * ECH], start=False, stop=False)
                        nc.tensor.matmul(out=pm, lhsT=w2_t[:, wb + 130:wb + 195],
                                         rhs=hgs[:, 0:ECH], start=False, stop=False)
                        nc.tensor.matmul(out=pm, lhsT=w2_t[:, wb + 195:wb + 260],
                                         rhs=hgs[:, ECH:2 * ECH], start=False, stop=True)
                        if c % 2 == 0:
                            nc.scalar.activation(out=mgT[:, rsl], in_=pm,
                                                 func=AF.Identity,
                                                 bias=b2_t[:, hl:hl + 1], scale=1.0)
                        else:
                            nc.vector.tensor_scalar(out=mgT[:, rsl], in0=pm,
                                                    scalar1=b2_t[:, hl:hl + 1],
                                                    scalar2=None, op0=OP.add)

                    # ---- attention smalls on [128, 125] ----
                    gsb = p_sm.tile([128, EPP], F32, name="gsb")
                    nc.sync.dma_start(
                        out=gsb,
                        in_=mgT[64:65, :].rearrange("o (p j) -> o p j", p=128))
                    wf = p_wf.tile([128, NPP], F32, name="wf")
                    nc.sync.dma_start(out=wf, in_=wfg[hl, sb])
                    t2 = p_sm.tile([128, EPP], F32, name="t2")
                    nc.scalar.activation(out=t2, in_=gsb, func=AF.Exp, scale=1.0)
                    t3 = p_sm.tile([128, EPP], F32, name="t3")
                    wf_ap = bass.AP(tensor=wf.tensor, offset=wf.offset,
                                    ap=[wf.ap[0], [5 * wf.ap[1][0], CPP],
                                        [0, 5], [wf.ap[1][0], 5]])
                    nc.vector.tensor_tensor(
                        out=t3.rearrange("p (a b c) -> p a b c", b=5, c=5),
                        in0=t2.rearrange("p (a b c) -> p a b c", b=5, c=5),
                        in1=wf_ap, op=OP.mult)
                    den = p_sm.tile([128, NPP], F32, name="den")
                    nc.vector.tensor_reduce(
                        out=den, in_=t3.rearrange("p (a b) -> p a b", b=5),
                        axis=mybir.AxisListType.X, op=OP.add)
                    rden = p_sm.tile([128, NPP], F32, name="rden")
                    nc.vector.tensor_scalar(out=rden, in0=den, scalar1=1e-10,
                                            scalar2=None, op0=OP.add)
                    nc.vector.reciprocal(out=rden, in_=rden)
                    coeff = p_sm.tile([128, EPP], F32R, name="coeff")
                    nc.vector.tensor_tensor(
                        out=coeff.rearrange("p (a b) -> p a b", b=5),
                        in0=t3.rearrange("p (a b) -> p a b", b=5),
                        in1=bcast_inner5(rden, NPP), op=OP.mult)
                    if sb == 0 and l == 0 and h == 0 and dbg_mg is not None:
                        nc.sync.dma_start(out=dbg_mg[:, :], in_=mgT)
                        nc.sync.dma_start(out=dbg_coeff[:, :], in_=coeff.bitcast(F32))
                    c4 = p_c4.tile([97, 4000], F32R, name="c4", tag="c4")
                    for k in range(4):
                        nc.sync.dma_start(
                            out=c4[32 * k:32 * k + 1, :].rearrange(
                                "o (p j) -> o p j", p=32),
                            in_=coeff[32 * k:32 * (k + 1), :])

                    # ---- apply coeff, segment-sum, update fea ----
                    for w in range(E_SB // EMSG_W):
                        em = p_em.tile([64, EMSG_W], F32, name="em")
                        for cc in range(EMSG_W // ECH):
                            c = w * (EMSG_W // ECH) + cc
                            k = (c * ECH) // 4000
                            koff = (c * ECH) % 4000
                            cb = ps_ms.tile([64, ECH], F32, name="cb", tag="ms")
                            nc.tensor.matmul(
                                out=cb, lhsT=ones_t[32 * k:32 * k + 1, :],
                                rhs=c4[32 * k:32 * k + 1, koff:koff + ECH],
                                start=True, stop=True, tile_position=(32 * k, 0))
                            nc.vector.tensor_tensor(
                                out=em[:, cc * ECH:(cc + 1) * ECH],
                                in0=mgT[0:64, c * ECH:(c + 1) * ECH],
                                in1=cb, op=OP.mult)
                        nodes = slice(w * (EMSG_W // 5), (w + 1) * (EMSG_W // 5))
                        u = p_em.tile([64, EMSG_W // 5], F32, name="u")
                        nc.vector.tensor_reduce(
                            out=u, in_=em.rearrange("p (a b) -> p a b", b=5),
                            axis=mybir.AxisListType.X, op=OP.add)
                        nc.vector.tensor_tensor(out=feaT[:, nodes],
                                                in0=feaT[:, nodes], in1=u,
                                                op=OP.add)

                if sb == 0 and l == 0 and dbg_fea1 is not None:
                    nc.sync.dma_start(out=dbg_fea1[:, :], in_=feaT.bitcast(F32))

            # ================= crystal pooling =================
            outT = p_out.tile([64, SB_CR], F32, name="outT")
            nc.vector.memset(outT, 0.0)
            for h in range(NH):
                mgc = p_mg.tile([65, N_SB], F32, name="mgc", tag="mgT")
                for c in range(NCRCH):
                    rsl = slice(c * CRCH, (c + 1) * CRCH)
                    hg = ps_h1.tile([128, 1024], F32, name="hg")
                    hm = ps_h1.tile([128, 1024], F32, name="hm")
                    for blk in range(2):
                        psl = slice(blk * 512, blk * 512 + CRCH)
                        wcol = ((h * 2 + 0) * 2 + blk) * 128
                        nc.tensor.matmul(out=hg[:, psl],
                                         lhsT=w1c_t[:, wcol:wcol + 128],
                                         rhs=feaT[:, rsl], start=True, stop=True)
                        wcol = ((h * 2 + 1) * 2 + blk) * 128
                        nc.tensor.matmul(out=hm[:, psl],
                                         lhsT=w1c_t[:, wcol:wcol + 128],
                                         rhs=feaT[:, rsl], start=True, stop=True)
                    hgs = p_h1.tile([128, 2 * CRCH], F32R, name="hgs")
                    hms = p_h1.tile([128, 2 * CRCH], F32R, name="hms")
                    for blk in range(2):
                        bsl = slice(blk * CRCH, (blk + 1) * CRCH)
                        psl = slice(blk * 512, blk * 512 + CRCH)
                        bg = (h * 2 + 0) * 2 + blk
                        bm = (h * 2 + 1) * 2 + blk
                        nc.scalar.activation(out=hgs[:, bsl], in_=hg[:, psl],
                                             func=AF.Lrelu, bias=b1c_t[:, bg:bg + 1],
                                             scale=1.0, alpha=0.01)
                        nc.scalar.activation(out=hms[:, bsl], in_=hm[:, psl],
                                             func=AF.Lrelu, bias=b1c_t[:, bm:bm + 1],
                                             scale=1.0, alpha=0.01)
                    pm = ps_mg.tile([65, CRCH], F32, name="pm")
                    wb = h * 4 * 65
                    nc.tensor.matmul(out=pm, lhsT=w2c_t[:, wb:wb + 65],
                                     rhs=hms[:, 0:CRCH], start=True, stop=False)
                    nc.tensor.matmul(out=pm, lhsT=w2c_t[:, wb + 65:wb + 130],
                                     rhs=hms[:, CRCH:2 * CRCH], start=False, stop=False)
                    nc.tensor.matmul(out=pm, lhsT=w2c_t[:, wb + 130:wb + 195],
                                     rhs=hgs[:, 0:CRCH], start=False, stop=False)
                    nc.tensor.matmul(out=pm, lhsT=w2c_t[:, wb + 195:wb + 260],
                                     rhs=hgs[:, CRCH:2 * CRCH], start=False, stop=True)
                    if c % 2 == 0:
                        nc.scalar.activation(out=mgc[:, rsl], in_=pm,
                                             func=AF.Identity,
                                             bias=b2_t[:, 9 + h:10 + h], scale=1.0)
                    else:
                        nc.vector.tensor_scalar(out=mgc[:, rsl], in0=pm,
                                                scalar1=b2_t[:, 9 + h:10 + h],
                                                scalar2=None, op0=OP.add)

                gsb = p_sm.tile([128, NPP], F32, name="gsbc")
                nc.sync.dma_start(
                    out=gsb, in_=mgc[64:65, :].rearrange("o (p j) -> o p j", p=128))
                wf = p_wf.tile([128, NPP], F32, name="wf")
                nc.sync.dma_start(out=wf, in_=wfc[h, sb])
                t2 = p_sm.tile([128, NPP], F32, name="t2c")
                nc.scalar.activation(out=t2, in_=gsb, func=AF.Exp, scale=1.0)
                t3 = p_sm.tile([128, NPP], F32, name="t3c")
                nc.vector.tensor_tensor(out=t3, in0=t2, in1=wf, op=OP.mult)
                den = p_sm.tile([128, CPP], F32, name="denc")
                nc.vector.tensor_reduce(
                    out=den, in_=t3.rearrange("p (a b) -> p a b", b=5),
                    axis=mybir.AxisListType.X, op=OP.add)
                rden = p_sm.tile([128, CPP], F32, name="rdenc")
                nc.vector.tensor_scalar(out=rden, in0=den, scalar1=1e-10,
                                        scalar2=None, op0=OP.add)
                nc.vector.reciprocal(out=rden, in_=rden)
                coeff = p_sm.tile([128, NPP], F32R, name="coeffc")
                nc.vector.tensor_tensor(
                    out=coeff.rearrange("p (a b) -> p a b", b=5),
                    in0=t3.rearrange("p (a b) -> p a b", b=5),
                    in1=bcast_inner5(rden, CPP), op=OP.mult)
                c4 = p_c4.tile([97, 800], F32R, name="c4c", tag="c4")
                for k in range(4):
                    nc.sync.dma_start(
                        out=c4[32 * k:32 * k + 1, :].rearrange(
                            "o (p j) -> o p j", p=32),
                        in_=coeff[32 * k:32 * (k + 1), :])
                for w in range(N_SB // CEMSG_W):
                    em = p_em.tile([64, CEMSG_W], F32, name="em")
                    for cc in range(CEMSG_W // CRCH):
                        c = w * (CEMSG_W // CRCH) + cc
                        k = (c * CRCH) // 800
                        koff = (c * CRCH) % 800
                        cb = ps_ms.tile([64, CRCH], F32, name="cb", tag="ms")
                        nc.tensor.matmul(
                            out=cb, lhsT=ones_t[32 * k:32 * k + 1, :],
                            rhs=c4[32 * k:32 * k + 1, koff:koff + CRCH],
                            start=True, stop=True, tile_position=(32 * k, 0))
                        nc.vector.tensor_tensor(
                            out=em[:, cc * CRCH:(cc + 1) * CRCH],
                            in0=mgc[0:64, c * CRCH:(c + 1) * CRCH],
                            in1=cb, op=OP.mult)
                    crs = slice(w * (CEMSG_W // 5), (w + 1) * (CEMSG_W // 5))
                    u = p_em.tile([64, CEMSG_W // 5], F32, name="u")
                    nc.vector.tensor_reduce(
                        out=u, in_=em.rearrange("p (a b) -> p a b", b=5),
                        axis=mybir.AxisListType.X, op=OP.add)
                    nc.vector.tensor_tensor(out=outT[:, crs], in0=outT[:, crs],
                                            in1=u, op=OP.add)

            # ================= output transpose + store =================
            osb = p_out.tile([128, 5 * 64], F32, name="osb")
            for tt in range(5):
                po = ps_ms.tile([128, 64], F32, name="po", tag="ms")
                nc.tensor.transpose(po, outT[:, tt * 128:(tt + 1) * 128],
                                    id_t[0:64, 0:64])
                nc.scalar.copy(out=osb[:, tt * 64:(tt + 1) * 64], in_=po)
            nc.sync.dma_start(
                out=out[sb * SB_CR:(sb + 1) * SB_CR, :].rearrange(
                    "(a b) f -> b a f", b=128),
                in_=osb.rearrange("p (a f) -> p a f", f=64))


def _wfac(w, pw):
    if pw > 0:
        return np.power(w, pw, dtype=np.float32)
    return (1.0 / (np.power(w, abs(pw), dtype=np.float32) + 1e-10)).astype(np.float32)


def _lrelu(x):
    return np.where(x >= 0, x, 0.01 * x)


def _sample_gate_shifts(ew_full, fea0_full, params):
    """Exact forward on a sample of crystals to bound per-head gate ranges.

    Returns (G[9], Gc[3]): per-head shifts = sampled min(gate) - 30.
    """
    S = 2048
    idx = np.linspace(0, C - 1, S).astype(np.int64)
    nodes = (idx[:, None] * NPC + np.arange(NPC)[None, :]).ravel()
    fea = fea0_full[nodes].astype(np.float32)          # (S*5, 64)
    wgt = ew_full[nodes].astype(np.float32)            # (S*5, 1)
    G = np.zeros(NG * NH, np.float32)
    Gc = np.zeros(NH, np.float32)
    fs = fea.reshape(S, NPC, FEA)
    for l in range(NG):
        upd = np.zeros_like(fs)
        for h in range(NH):
            p = params["graphs"][l][h]
            W1g = np.asarray(p["gate"]["hidden"][0]["W"], np.float32)
            b1g = np.asarray(p["gate"]["hidden"][0]["b"], np.float32)
            W2g = np.asarray(p["gate"]["out"]["W"], np.float32)
            b2g = np.asarray(p["gate"]["out"]["b"], np.float32)
            W1m = np.asarray(p["msg"]["hidden"][0]["W"], np.float32)
            b1m = np.asarray(p["msg"]["hidden"][0]["b"], np.float32)
            W2m = np.asarray(p["msg"]["out"]["W"], np.float32)
            b2m = np.asarray(p["msg"]["out"]["b"], np.float32)
            pw = float(np.asarray(p["pow"])[0])
            cat = np.concatenate(
                [np.repeat(fs, NPC, axis=1),
                 np.tile(fs, (1, NPC, 1))], axis=2).reshape(S * 25, 2 * FEA)
            gate = (_lrelu(cat @ W1g + b1g) @ W2g).ravel()  # no b2g (cancels)
            G[l * NH + h] = gate.min()
            gate = gate.reshape(S, NPC, NPC)
            gmax = gate.max(axis=2, keepdims=True)
            wf = _wfac(wgt.reshape(S, NPC)[:, None, :], pw)
            e = wf * np.exp(gate - gmax)
            coef = e / (e.sum(axis=2, keepdims=True) + 1e-10)
            msg = (_lrelu(cat @ W1m + b1m) @ W2m + b2m).reshape(S, NPC, NPC, FEA)
            upd += (coef[..., None] * msg).sum(axis=2) / NH
        fs = fs + upd
    for h in range(NH):
        p = params["cry"][h]
        W1g = np.asarray(p["gate"]["hidden"][0]["W"], np.float32)
        b1g = np.asarray(p["gate"]["hidden"][0]["b"], np.float32)
        W2g = np.asarray(p["gate"]["out"]["W"], np.float32)
        x = fs.reshape(S * NPC, FEA)
        gate = (_lrelu(x @ W1g + b1g) @ W2g).ravel()
        Gc[h] = gate.min()
    return G - 30.0, Gc - 30.0  # shift; drain bias = -G


def _pack_inputs(elem_weights, elem_fea, params):
    ew_full = np.asarray(elem_weights, np.float32)            # (N,1)
    emb_W = np.asarray(params["embedding"]["W"], np.float32)  # (200,63)
    emb_b = np.asarray(params["embedding"]["b"], np.float32)
    ef_full = np.asarray(elem_fea, np.float32)

    # fea0 on host only for the gate-shift sample
    fea0 = ef_full @ emb_W + emb_b
    fea0 = np.concatenate([fea0, ew_full], axis=1)
    G, Gc = _sample_gate_shifts(ew_full, fea0, params)

    w1 = np.zeros((128, 36 * 128), np.float32)
    w2 = np.zeros((128, 36 * 65), np.float32)
    b1 = np.zeros((128, 36), np.float32)
    b2 = np.zeros((65, 12), np.float32)
    for l in range(NG):
        for h in range(NH):
            hl = l * NH + h
            p = params["graphs"][l][h]
            for net, key in ((0, "gate"), (1, "msg")):
                W1 = np.asarray(p[key]["hidden"][0]["W"], np.float32)
                B1 = np.asarray(p[key]["hidden"][0]["b"], np.float32)
                for blk in range(2):
                    i = (hl * 2 + net) * 2 + blk
                    w1[:, i * 128:(i + 1) * 128] = W1[:, blk * 128:(blk + 1) * 128]
                    b1[:, i] = B1[blk * 128:(blk + 1) * 128]
            W2g = np.asarray(p["gate"]["out"]["W"], np.float32)
            W2m = np.asarray(p["msg"]["out"]["W"], np.float32)
            B2m = np.asarray(p["msg"]["out"]["b"], np.float32)
            # blocks 0,1 = msg halves; 2,3 = gate halves
            for blk in range(2):
                a = np.zeros((128, 65), np.float32)
                a[:, 0:64] = W2m[blk * 128:(blk + 1) * 128, :] / NH
                w2[:, (hl * 4 + blk) * 65:(hl * 4 + blk + 1) * 65] = a
                g = np.zeros((128, 65), np.float32)
                g[:, 64] = W2g[blk * 128:(blk + 1) * 128, 0]
                w2[:, (hl * 4 + 2 + blk) * 65:(hl * 4 + 3 + blk) * 65] = g
            b2[0:64, hl] = B2m / NH
            b2[64, hl] = -G[hl]

    w1c = np.zeros((64, 12 * 128), np.float32)
    w2c = np.zeros((128, 12 * 65), np.float32)
    b1c = np.zeros((128, 12), np.float32)
    for h in range(NH):
        p = params["cry"][h]
        for net, key in ((0, "gate"), (1, "msg")):
            W1 = np.asarray(p[key]["hidden"][0]["W"], np.float32)
            B1 = np.asarray(p[key]["hidden"][0]["b"], np.float32)
            for blk in range(2):
                i = (h * 2 + net) * 2 + blk
                w1c[:, i * 128:(i + 1) * 128] = W1[:, blk * 128:(blk + 1) * 128]
                b1c[:, i] = B1[blk * 128:(blk + 1) * 128]
        W2g = np.asarray(p["gate"]["out"]["W"], np.float32)
        W2m = np.asarray(p["msg"]["out"]["W"], np.float32)
        B2m = np.asarray(p["msg"]["out"]["b"], np.float32)
        for blk in range(2):
            a = np.zeros((128, 65), np.float32)
            a[:, 0:64] = W2m[blk * 128:(blk + 1) * 128, :] / NH
            w2c[:, (h * 4 + blk) * 65:(h * 4 + blk + 1) * 65] = a
            g = np.zeros((128, 65), np.float32)
            g[:, 64] = W2g[blk * 128:(blk + 1) * 128, 0]
            w2c[:, (h * 4 + 2 + blk) * 65:(h * 4 + 3 + blk) * 65] = g
        b2[0:64, 9 + h] = B2m / NH
        b2[64, 9 + h] = -Gc[h]

    shared = {
        "w1": w1.astype(BF), "w2": w2, "b1": b1, "b2": b2,
        "w1c": w1c, "w2c": w2c, "b1c": b1c,
        "wemba": np.ascontiguousarray(emb_W[0:128, :]),
        "wembb": np.ascontiguousarray(emb_W[128:200, :]),
        "bemb": emb_b.reshape(63, 1).astype(np.float32),
        "ident": np.eye(128, dtype=np.float32),
    }

    # per-head per-node wfac, per core
    in_maps = []
    for k in range(NCORES):
        n0 = k * N_CORE
        ewk = ew_full[n0:n0 + N_CORE, 0]
        wfg_k = np.zeros((NG * NH, NSB, 128, NPP), np.float32)
        wfc_k = np.zeros((NH, NSB, 128, NPP), np.float32)
        for l in range(NG):
            for h in range(NH):
                pw = float(np.asarray(params["graphs"][l][h]["pow"])[0])
                wfg_k[l * NH + h] = _wfac(ewk, pw).reshape(NSB, 128, NPP)
        for h in range(NH):
            pw = float(np.asarray(params["cry"][h]["pow"])[0])
            wfc_k[h] = _wfac(ewk, pw).reshape(NSB, 128, NPP)
        m = dict(shared)
        m["ef"] = np.ascontiguousarray(ef_full[n0:n0 + N_CORE])
        m["ew"] = np.ascontiguousarray(ew_full[n0:n0 + N_CORE, 0]).reshape(1, N_CORE)
        m["wfg"] = wfg_k
        m["wfc"] = wfc_k
        in_maps.append(m)
    return in_maps


def _check_idx(self_fea_idx, nbr_fea_idx, cry_elem_idx):
    nodes = np.arange(C * NPC, dtype=np.int64).reshape(C, NPC)
    ok = (np.array_equal(np.asarray(self_fea_idx).ravel(),
                         np.repeat(nodes, NPC, axis=1).ravel())
          and np.array_equal(np.asarray(nbr_fea_idx).ravel(),
                             np.tile(nodes, (1, NPC)).ravel())
          and np.array_equal(np.asarray(cry_elem_idx).ravel(),
                             np.repeat(np.arange(C, dtype=np.int64), NPC)))
    if not ok:
        raise ValueError("index inputs do not match the expected crystal structure")


def kernel(elem_weights, elem_fea, self_fea_idx, nbr_fea_idx, cry_elem_idx, params):
    import os
    _check_idx(self_fea_idx, nbr_fea_idx, cry_elem_idx)
    if "nc" not in _CACHE:
        _CACHE["nc"] = _build()
    nc = _CACHE["nc"]
    in_maps = _pack_inputs(elem_weights, elem_fea, params)
    trace = os.environ.get("KERNEL_TRACE") == "1"
    res = run_bass_kernel_spmd(nc, in_maps, core_ids=list(range(NCORES)),
                               trace=trace)
    _CACHE["last"] = res
    return np.concatenate([r["out"] for r in res.results], axis=0)


# revision 15
# speedup vs baseline: 3.4795x; 3.4795x over previous
"""Trainium2 Bass kernel for nn_DescriptorNetwork (gnn_message_passing).

Strategy:
 - Shard by crystal: core k owns crystals [3200k, 3200(k+1)) -> nodes/edges
   contiguous. Params replicated. All gathers are crystal-local.
 - Feature-major on-chip layout: feaT [64, nodes]; per-superblock streaming
   (640 crystals = 3200 nodes = 16000 edges per SB, 5 SBs per core).
 - Hidden layers of gate/msg nets: bf16 matmuls on catT [128, edges].
   cat gather = AP tricks (each crystal fully connected, 5 nodes).
 - gate output row rides the msg matmul as a 65th lhsT column -> one
   [65, chunk] PSUM tile, one ACT/DVE drain pass.
 - Attention softmax on [128, 125] tiles (partition p owns 5 crystals),
   reached via SBUF->SBUF reshape DMA of the gate row.
 - exp stabilization: constant per-head shift G (baked into drain bias),
   valid because softmax is shift-invariant; G chosen from an exact
   host-side forward of a sample of crystals so exp stays in range.
 - coeff broadcast across 64 feature partitions via PE outer product
   (ones[1,64] x coeff-row), PSUM-accumulation-free.
 - segment sums (5 edges/node, 5 nodes/crystal) = strided tensor_reduce.
"""
import sys
sys.path.insert(0, "/opt/trn_rl_repo")

import numpy as np
import ml_dtypes

import concourse.bass as bass
import concourse.bacc as bacc
import concourse.tile as tile
import concourse.mybir as mybir
from concourse.bass_utils import run_bass_kernel_spmd

F32 = mybir.dt.float32
F32R = mybir.dt.float32r
BF16 = mybir.dt.bfloat16
AF = mybir.ActivationFunctionType
OP = mybir.AluOpType
BF = ml_dtypes.bfloat16

NCORES = 8
C = 25600
NPC = 5
FEA = 64
EMB = 200
HID = 256
NG = 3          # graph layers
NH = 3          # heads per layer / cry heads
C_CORE = C // NCORES            # 3200 crystals
N_CORE = C_CORE * NPC           # 16000 nodes
E_CORE = N_CORE * NPC           # 80000 edges
SB_CR = 640                     # crystals per superblock
NSB = C_CORE // SB_CR           # 5
N_SB = SB_CR * NPC              # 3200 nodes / SB
E_SB = N_SB * NPC               # 16000 edges / SB
ECH = 500                       # edge chunk (<=512 psum cols)
NECH = E_SB // ECH              # 32
CRCH = 400                      # cry node chunk
NCRCH = N_SB // CRCH            # 8
EPP = E_SB // 128               # 125 edges per partition (5 crystals)
NPP = N_SB // 128               # 25 nodes per partition
CPP = SB_CR // 128              # 5 crystals per partition
NGRP = 640                      # emb node group
EMSG_W = 1000                   # emsg window (edges)
CEMSG_W = 800                   # cry emsg window (nodes)

_CACHE = {}


def _build():
    nc = bacc.Bacc(None, target_bir_lowering=False)

    ef = nc.dram_tensor("ef", [N_CORE, EMB], F32, kind="ExternalInput")
    ew = nc.dram_tensor("ew", [1, N_CORE], F32R, kind="ExternalInput")
    wfg = nc.dram_tensor("wfg", [NG * NH, NSB, 128, NPP], F32, kind="ExternalInput")
    wfc = nc.dram_tensor("wfc", [NH, NSB, 128, NPP], F32, kind="ExternalInput")
    w1 = nc.dram_tensor("w1", [128, 36 * 128], BF16, kind="ExternalInput")
    w2 = nc.dram_tensor("w2", [128, 36 * 65], F32, kind="ExternalInput")
    b1 = nc.dram_tensor("b1", [128, 36], F32, kind="ExternalInput")
    b2 = nc.dram_tensor("b2", [65, 12], F32, kind="ExternalInput")
    w1c = nc.dram_tensor("w1c", [64, 12 * 128], F32, kind="ExternalInput")
    w2c = nc.dram_tensor("w2c", [128, 12 * 65], F32, kind="ExternalInput")
    b1c = nc.dram_tensor("b1c", [128, 12], F32, kind="ExternalInput")
    wemba = nc.dram_tensor("wemba", [128, 63], F32, kind="ExternalInput")
    wembb = nc.dram_tensor("wembb", [72, 63], F32, kind="ExternalInput")
    bemb = nc.dram_tensor("bemb", [63, 1], F32, kind="ExternalInput")
    ident = nc.dram_tensor("ident", [128, 128], F32, kind="ExternalInput")
    out = nc.dram_tensor("out", [C_CORE, FEA], F32, kind="ExternalOutput")
    import os
    if os.environ.get("KERNEL_DEBUG") == "1":
        dbg_fea = nc.dram_tensor("dbg_fea", [64, N_SB], F32, kind="ExternalOutput")
        dbg_mg = nc.dram_tensor("dbg_mg", [65, E_SB], F32, kind="ExternalOutput")
        dbg_coeff = nc.dram_tensor("dbg_coeff", [128, EPP], F32, kind="ExternalOutput")
        dbg_fea1 = nc.dram_tensor("dbg_fea1", [64, N_SB], F32, kind="ExternalOutput")
    else:
        dbg_fea = dbg_mg = dbg_coeff = dbg_fea1 = None

    with tile.TileContext(nc) as tc:
        _emit(nc, tc, locals())
    nc.finalize()
    return nc


def _emit(nc, tc, t):
    ef, ew, wfg, wfc = t["ef"], t["ew"], t["wfg"], t["wfc"]
    w1, w2, b1, b2 = t["w1"], t["w2"], t["b1"], t["b2"]
    w1c, w2c, b1c = t["w1c"], t["w2c"], t["b1c"]
    wemba, wembb, bemb, ident, out = t["wemba"], t["wembb"], t["bemb"], t["ident"], t["out"]
    dbg_fea, dbg_mg, dbg_coeff, dbg_fea1 = t["dbg_fea"], t["dbg_mg"], t["dbg_coeff"], t["dbg_fea1"]

    import contextlib
    ctx = contextlib.ExitStack()
    with ctx:
        const = ctx.enter_context(tc.tile_pool(name="const", bufs=1))
        p_fea = ctx.enter_context(tc.tile_pool(name="p_fea", bufs=1))
        p_cat = ctx.enter_context(tc.tile_pool(name="p_cat", bufs=1))
        p_mg = ctx.enter_context(tc.tile_pool(name="p_mg", bufs=1))
        p_h1 = ctx.enter_context(tc.tile_pool(name="p_h1", bufs=2))
        p_sm = ctx.enter_context(tc.tile_pool(name="p_sm", bufs=2))
        p_c4 = ctx.enter_context(tc.tile_pool(name="p_c4", bufs=1))
        p_em = ctx.enter_context(tc.tile_pool(name="p_em", bufs=2))
        p_ef = ctx.enter_context(tc.tile_pool(name="p_ef", bufs=3))
        p_eft = ctx.enter_context(tc.tile_pool(name="p_eft", bufs=1))
        p_out = ctx.enter_context(tc.tile_pool(name="p_out", bufs=1))
        p_wf = ctx.enter_context(tc.tile_pool(name="p_wf", bufs=3))

        ps_h1 = ctx.enter_context(tc.tile_pool(name="ps_h1", bufs=1, space="PSUM"))
        ps_mg = ctx.enter_context(tc.tile_pool(name="ps_mg", bufs=2, space="PSUM"))
        ps_ms = ctx.enter_context(tc.tile_pool(name="ps_ms", bufs=2, space="PSUM"))

        # ---- resident params ----
        w1_t = const.tile([128, 36 * 128], BF16, name="w1t")
        nc.sync.dma_start(out=w1_t, in_=w1[:, :])
        b1_t = const.tile([128, 36], F32, name="b1t")
        nc.sync.dma_start(out=b1_t, in_=b1[:, :])
        b2_t = const.tile([65, 12], F32, name="b2t")
        nc.sync.dma_start(out=b2_t, in_=b2[:, :])
        b1c_t = const.tile([128, 12], F32, name="b1ct")
        nc.sync.dma_start(out=b1c_t, in_=b1c[:, :])
        bemb_t = const.tile([63, 1], F32, name="bembt")
        nc.sync.dma_start(out=bemb_t, in_=bemb[:, :])
        id_t = const.tile([128, 128], F32, name="idt")
        nc.sync.dma_start(out=id_t, in_=ident[:, :])

        # f32r params go through a DVE copy (producer must round to f32r)
        w2_t = const.tile([128, 36 * 65], F32R, name="w2t")
        w1c_t = const.tile([64, 12 * 128], F32R, name="w1ct")
        w2c_t = const.tile([128, 12 * 65], F32R, name="w2ct")
        wea_t = const.tile([128, 63], F32R, name="weat")
        web_t = const.tile([72, 63], F32R, name="webt")
        with tc.tile_pool(name="stg", bufs=1) as stg:
            for (dst, srcd, shp) in ((w2_t, w2, [128, 36 * 65]),
                                     (w1c_t, w1c, [64, 12 * 128]),
                                     (w2c_t, w2c, [128, 12 * 65]),
                                     (wea_t, wemba, [128, 63]),
                                     (web_t, wembb, [72, 63])):
                for c0 in range(0, shp[1], 640):
                    c1 = min(c0 + 640, shp[1])
                    s = stg.tile([128, 640], F32, name="stg_s", tag="s")
                    nc.sync.dma_start(out=s[0:shp[0], 0:c1 - c0],
                                      in_=srcd[:, c0:c1])
                    nc.vector.tensor_copy(out=dst[:, c0:c1],
                                          in_=s[0:shp[0], 0:c1 - c0])
        ones_s = const.tile([128, 64], F32, name="oness")
        nc.vector.memset(ones_s, 1.0)
        ones_t = const.tile([128, 64], F32R, name="onest")
        nc.vector.tensor_copy(out=ones_t, in_=ones_s)

        def bcast_inner5(ap2d, n):
            """[P, n] -> [P, n, 5] with 0-step inner (broadcast each col 5x)."""
            return bass.AP(tensor=ap2d.tensor, offset=ap2d.offset,
                           ap=[ap2d.ap[0], [ap2d.ap[1][0], n], [0, 5]])

        import os as _os
        reps = int(_os.environ.get("KERNEL_REPS", "1"))
        for sb in [s for r in range(reps) for s in range(NSB)]:
            nbase = sb * N_SB

            # ================= embedding =================
            feaT = p_fea.tile([64, N_SB], F32R, name="feaT")
            for g in range(N_SB // NGRP):
                hi = p_eft.tile([128, NGRP], F32R, name="efhi")
                lo = p_eft.tile([72, NGRP], F32R, name="eflo")
                rows = nbase + g * NGRP
                eft = p_ef.tile([128, 5 * EMB], F32, name="eft")
                nc.sync.dma_start(
                    out=eft.rearrange("p (t f) -> p t f", t=5),
                    in_=ef[rows:rows + NGRP, :].rearrange("(t p) f -> p t f", p=128))
                for tt in range(NGRP // 128):
                    ph = ps_ms.tile([128, 128], F32, name="pst", tag="ms")
                    nc.tensor.transpose(ph, eft[:, tt * EMB:tt * EMB + 128], id_t)
                    nc.vector.tensor_copy(out=hi[:, tt * 128:(tt + 1) * 128], in_=ph)
                    pl = ps_ms.tile([72, 128], F32, name="psl", tag="ms")
                    nc.tensor.transpose(pl, eft[:, tt * EMB + 128:tt * EMB + 200], id_t)
                    nc.vector.tensor_copy(out=lo[:, tt * 128:(tt + 1) * 128], in_=pl)
                for half in range(2):
                    cols = slice(half * (NGRP // 2), (half + 1) * (NGRP // 2))
                    pe = ps_ms.tile([63, NGRP // 2], F32, name="pse", tag="ms")
                    nc.tensor.matmul(out=pe, lhsT=wea_t, rhs=hi[:, cols],
                                     start=True, stop=False)
                    nc.tensor.matmul(out=pe, lhsT=web_t, rhs=lo[:, cols],
                                     start=False, stop=True)
                    dst = slice(g * NGRP + half * (NGRP // 2),
                                g * NGRP + (half + 1) * (NGRP // 2))
                    nc.scalar.activation(out=feaT[0:63, dst], in_=pe,
                                         func=AF.Identity, bias=bemb_t, scale=1.0)
            nc.sync.dma_start(out=feaT[63:64, :],
                              in_=ew[0:1, nbase:nbase + N_SB])
            if sb == 0 and dbg_fea is not None:
                nc.sync.dma_start(out=dbg_fea[:, :], in_=feaT.bitcast(F32))

            # ================= graph layers =================
            for l in range(NG):
                catq = [p_cat.tile([128, E_SB // 4], BF16, name=f"catT{q}",
                                   tag=f"catT{q}") for q in range(4)]
                for c in range(NECH):
                    crys = slice((c * ECH) // 25 * 5, ((c + 1) * ECH) // 25 * 5)
                    # self: node = 5cr + i, repeated 5x ; nbr: tile of 5
                    base = feaT[:, crys]
                    self_ap = bass.AP(tensor=base.tensor, offset=base.offset,
                                      ap=[base.ap[0], [5, ECH // 25], [1, 5], [0, 5]])
                    nbr_ap = bass.AP(tensor=base.tensor, offset=base.offset,
                                     ap=[base.ap[0], [5, ECH // 25], [0, 5], [1, 5]])
                    oc = catq[c // 8][:, (c % 8) * ECH:(c % 8 + 1) * ECH]
                    o3 = oc.rearrange("p (a b c) -> p a b c", b=5, c=5)
                    nc.gpsimd.tensor_copy(out=o3[0:64], in_=self_ap)
                    nc.gpsimd.tensor_copy(out=o3[64:128], in_=nbr_ap)

                for h in range(NH):
                    hl = l * NH + h
                    mgq = [p_mg.tile([65, E_SB // 4], F32, name=f"mg{q}",
                                     tag=f"mg{q}") for q in range(4)]
                    for c in range(NECH):
                        hg = ps_h1.tile([128, 1024], F32, name="hg")
                        hm = ps_h1.tile([128, 1024], F32, name="hm")
                        crhs = catq[c // 8][:, (c % 8) * ECH:(c % 8 + 1) * ECH]
                        for blk in range(2):
                            psl = slice(blk * 512, blk * 512 + ECH)
                            wcol = ((hl * 2 + 0) * 2 + blk) * 128
                            nc.tensor.matmul(out=hg[:, psl],
                                             lhsT=w1_t[:, wcol:wcol + 128],
                                             rhs=crhs, start=True, stop=True)
                            wcol = ((hl * 2 + 1) * 2 + blk) * 128
                            nc.tensor.matmul(out=hm[:, psl],
                                             lhsT=w1_t[:, wcol:wcol + 128],
                                             rhs=crhs, start=True, stop=True)
                        hgs = p_h1.tile([128, 2 * ECH], F32R, name="hgs")
                        hms = p_h1.tile([128, 2 * ECH], F32R, name="hms")
                        for blk in range(2):
                            bsl = slice(blk * ECH, (blk + 1) * ECH)
                            psl = slice(blk * 512, blk * 512 + ECH)
                            bg = (hl * 2 + 0) * 2 + blk
                            bm = (hl * 2 + 1) * 2 + blk
                            nc.scalar.activation(out=hgs[:, bsl], in_=hg[:, psl],
                                                 func=AF.Lrelu,
                                                 bias=b1_t[:, bg:bg + 1],
                                                 scale=1.0, alpha=0.01)
                            nc.scalar.activation(out=hms[:, bsl], in_=hm[:, psl],
                                                 func=AF.Lrelu,
                                                 bias=b1_t[:, bm:bm + 1],
                                                 scale=1.0, alpha=0.01)
                        pm = ps_mg.tile([65, ECH], F32, name="pm")
                        wb = hl * 4 * 65
                        nc.tensor.matmul(out=pm, lhsT=w2_t[:, wb:wb + 65],
                                         rhs=hms[:, 0:ECH], start=True, stop=False)
                        nc.tensor.matmul(out=pm, lhsT=w2_t[:, wb + 65:wb + 130],
                                         rhs=hms[:, ECH:2 * ECH], start=False, stop=False)
                        nc.tensor.matmul(out=pm, lhsT=w2_t[:, wb + 130:wb + 195],
                                         rhs=hgs[:, 0:ECH], start=False, stop=False)
                        nc.tensor.matmul(out=pm, lhsT=w2_t[:, wb + 195:wb + 260],
                                         rhs=hgs[:, ECH:2 * ECH], start=False, stop=True)
                        nc.vector.tensor_scalar(
                            out=mgq[c // 8][:, (c % 8) * ECH:(c % 8 + 1) * ECH],
                            in0=pm, scalar1=b2_t[:, hl:hl + 1],
                            scalar2=None, op0=OP.add)

                    # ---- attention smalls on [128, 125] ----
                    gsb = p_sm.tile([128, EPP], F32, name="gsb")
                    for q in range(4):
                        nc.sync.dma_start(
                            out=gsb[32 * q:32 * (q + 1), :],
                            in_=mgq[q][64:65, :].rearrange("o (p j) -> o p j", p=32))
                    wf = p_wf.tile([128, NPP], F32, name="wf")
                    nc.sync.dma_start(out=wf, in_=wfg[hl, sb])
                    t2 = p_sm.tile([128, EPP], F32, name="t2")
                    nc.scalar.activation(out=t2, in_=gsb, func=AF.Exp, scale=1.0)
                    t3 = p_sm.tile([128, EPP], F32, name="t3")
                    wf_ap = bass.AP(tensor=wf.tensor, offset=wf.offset,
                                    ap=[wf.ap[0], [5 * wf.ap[1][0], CPP],
                                        [0, 5], [wf.ap[1][0], 5]])
                    nc.vector.tensor_tensor(
                        out=t3.rearrange("p (a b c) -> p a b c", b=5, c=5),
                        in0=t2.rearrange("p (a b c) -> p a b c", b=5, c=5),
                        in1=wf_ap, op=OP.mult)
                    den = p_sm.tile([128, NPP], F32, name="den")
                    nc.vector.tensor_reduce(
                        out=den, in_=t3.rearrange("p (a b) -> p a b", b=5),
                        axis=mybir.AxisListType.X, op=OP.add)
                    rden = p_sm.tile([128, NPP], F32, name="rden")
                    nc.vector.tensor_scalar(out=rden, in0=den, scalar1=1e-10,
                                            scalar2=None, op0=OP.add)
                    nc.vector.reciprocal(out=rden, in_=rden)
                    coeff = p_sm.tile([128, EPP], F32R, name="coeff")
                    nc.vector.tensor_tensor(
                        out=coeff.rearrange("p (a b) -> p a b", b=5),
                        in0=t3.rearrange("p (a b) -> p a b", b=5),
                        in1=bcast_inner5(rden, NPP), op=OP.mult)
                    if sb == 0 and l == 0 and h == 0 and dbg_mg is not None:
                        nc.sync.dma_start(out=dbg_mg[:, :], in_=mgT)
                        nc.sync.dma_start(out=dbg_coeff[:, :], in_=coeff.bitcast(F32))
                    c4 = p_c4.tile([97, 4000], F32R, name="c4", tag="c4")
                    for k in range(4):
                        nc.sync.dma_start(
                            out=c4[32 * k:32 * k + 1, :].rearrange(
                                "o (p j) -> o p j", p=32),
                            in_=coeff[32 * k:32 * (k + 1), :])

                    # ---- apply coeff, segment-sum, update fea ----
                    for w in range(E_SB // EMSG_W):
                        em = p_em.tile([64, EMSG_W], F32, name="em")
                        for cc in range(EMSG_W // ECH):
                            c = w * (EMSG_W // ECH) + cc
                            k = (c * ECH) // 4000
                            koff = (c * ECH) % 4000
                            cb = ps_ms.tile([64, ECH], F32, name="cb", tag="ms")
                            nc.tensor.matmul(
                                out=cb, lhsT=ones_t[32 * k:32 * k + 1, :],
                                rhs=c4[32 * k:32 * k + 1, koff:koff + ECH],
                                start=True, stop=True, tile_position=(32 * k, 0))
                            nc.vector.tensor_tensor(
                                out=em[:, cc * ECH:(cc + 1) * ECH],
                                in0=mgq[c // 8][0:64, (c % 8) * ECH:(c % 8 + 1) * ECH],
                                in1=cb, op=OP.mult)
                        nodes = slice(w * (EMSG_W // 5), (w + 1) * (EMSG_W // 5))
                        u = p_em.tile([64, EMSG_W // 5], F32, name="u")
                        nc.vector.tensor_reduce(
                            out=u, in_=em.rearrange("p (a b) -> p a b", b=5),
                            axis=mybir.AxisListType.X, op=OP.add)
                        nc.vector.tensor_tensor(out=feaT[:, nodes],
                                                in0=feaT[:, nodes], in1=u,
                                                op=OP.add)

                if sb == 0 and l == 0 and dbg_fea1 is not None:
                    nc.sync.dma_start(out=dbg_fea1[:, :], in_=feaT.bitcast(F32))

            # ================= crystal pooling =================
            outT = p_out.tile([64, SB_CR], F32, name="outT")
            nc.vector.memset(outT, 0.0)
            for h in range(NH):
                mgcq = [p_mg.tile([65, N_SB // 4], F32, name=f"mgc{q}",
                                  tag=f"mg{q}") for q in range(4)]
                for c in range(NCRCH):
                    rsl = slice(c * CRCH, (c + 1) * CRCH)
                    hg = ps_h1.tile([128, 1024], F32, name="hg")
                    hm = ps_h1.tile([128, 1024], F32, name="hm")
                    for blk in range(2):
                        psl = slice(blk * 512, blk * 512 + CRCH)
                        wcol = ((h * 2 + 0) * 2 + blk) * 128
                        nc.tensor.matmul(out=hg[:, psl],
                                         lhsT=w1c_t[:, wcol:wcol + 128],
                                         rhs=feaT[:, rsl], start=True, stop=True)
                        wcol = ((h * 2 + 1) * 2 + blk) * 128
                        nc.tensor.matmul(out=hm[:, psl],
                                         lhsT=w1c_t[:, wcol:wcol + 128],
                                         rhs=feaT[:, rsl], start=True, stop=True)
                    hgs = p_h1.tile([128, 2 * CRCH], F32R, name="hgs")
                    hms = p_h1.tile([128, 2 * CRCH], F32R, name="hms")
                    for blk in range(2):
                        bsl = slice(blk * CRCH, (blk + 1) * CRCH)
                        psl = slice(blk * 512, blk * 512 + CRCH)
                        bg = (h * 2 + 0) * 2 + blk
                        bm = (h * 2 + 1) * 2 + blk
                        nc.scalar.activation(out=hgs[:, bsl], in_=hg[:, psl],
                                             func=AF.Lrelu, bias=b1c_t[:, bg:bg + 1],
                                             scale=1.0, alpha=0.01)
                        nc.scalar.activation(out=hms[:, bsl], in_=hm[:, psl],
                                             func=AF.Lrelu, bias=b1c_t[:, bm:bm + 1],
                                             scale=1.0, alpha=0.01)
                    pm = ps_mg.tile([65, CRCH], F32, name="pm")
                    wb = h * 4 * 65
                    nc.tensor.matmul(out=pm, lhsT=w2c_t[:, wb:wb + 65],
                                     rhs=hms[:, 0:CRCH], start=True, stop=False)
                    nc.tensor.matmul(out=pm, lhsT=w2c_t[:, wb + 65:wb + 130],
                                     rhs=hms[:, CRCH:2 * CRCH], start=False, stop=False)
                    nc.tensor.matmul(out=pm, lhsT=w2c_t[:, wb + 130:wb + 195],
                                     rhs=hgs[:, 0:CRCH], start=False, stop=False)
                    nc.tensor.matmul(out=pm, lhsT=w2c_t[:, wb + 195:wb + 260],
                                     rhs=hgs[:, CRCH:2 * CRCH], start=False, stop=True)
                    nc.vector.tensor_scalar(
                        out=mgcq[c // 2][:, (c % 2) * CRCH:(c % 2 + 1) * CRCH],
                        in0=pm, scalar1=b2_t[:, 9 + h:10 + h],
                        scalar2=None, op0=OP.add)

                gsb = p_sm.tile([128, NPP], F32, name="gsbc")
                for q in range(4):
                    nc.sync.dma_start(
                        out=gsb[32 * q:32 * (q + 1), :],
                        in_=mgcq[q][64:65, :].rearrange("o (p j) -> o p j", p=32))
                wf = p_wf.tile([128, NPP], F32, name="wf")
                nc.sync.dma_start(out=wf, in_=wfc[h, sb])
                t2 = p_sm.tile([128, NPP], F32, name="t2c")
                nc.scalar.activation(out=t2, in_=gsb, func=AF.Exp, scale=1.0)
                t3 = p_sm.tile([128, NPP], F32, name="t3c")
                nc.vector.tensor_tensor(out=t3, in0=t2, in1=wf, op=OP.mult)
                den = p_sm.tile([128, CPP], F32, name="denc")
                nc.vector.tensor_reduce(
                    out=den, in_=t3.rearrange("p (a b) -> p a b", b=5),
                    axis=mybir.AxisListType.X, op=OP.add)
                rden = p_sm.tile([128, CPP], F32, name="rdenc")
                nc.vector.tensor_scalar(out=rden, in0=den, scalar1=1e-10,
                                        scalar2=None, op0=OP.add)
                nc.vector.reciprocal(out=rden, in_=rden)
                coeff = p_sm.tile([128, NPP], F32R, name="coeffc")
                nc.vector.tensor_tensor(
                    out=coeff.rearrange("p (a b) -> p a b", b=5),
                    in0=t3.rearrange("p (a b) -> p a b", b=5),
                    in1=bcast_inner5(rden, CPP), op=OP.mult)
                c4 = p_c4.tile([97, 800], F32R, name="c4c", tag="c4")
                for k in range(4):
                    nc.sync.dma_start(
                        out=c4[32 * k:32 * k + 1, :].rearrange(
                            "o (p j) -> o p j", p=32),
                        in_=coeff[32 * k:32 * (k + 1), :])
                for w in range(N_SB // CEMSG_W):
                    em = p_em.tile([64, CEMSG_W], F32, name="em")
                    for cc in range(CEMSG_W // CRCH):
                        c = w * (CEMSG_W // CRCH) + cc
                        k = (c * CRCH) // 800
                        koff = (c * CRCH) % 800
                        cb = ps_ms.tile([64, CRCH], F32, name="cb", tag="ms")
                        nc.tensor.matmul(
                            out=cb, lhsT=ones_t[32 * k:32 * k + 1, :],
                            rhs=c4[32 * k:32 * k + 1, koff:koff + CRCH],
                            start=True, stop=True, tile_position=(32 * k, 0))
                        nc.vector.tensor_tensor(
                            out=em[:, cc * CRCH:(cc + 1) * CRCH],
                            in0=mgcq[c // 2][0:64, (c % 2) * CRCH:(c % 2 + 1) * CRCH],
                            in1=cb, op=OP.mult)
                    crs = slice(w * (CEMSG_W // 5), (w + 1) * (CEMSG_W // 5))
                    u = p_em.tile([64, CEMSG_W // 5], F32, name="u")
                    nc.vector.tensor_reduce(
                        out=u, in_=em.rearrange("p (a b) -> p a b", b=5),
                        axis=mybir.AxisListType.X, op=OP.add)
                    nc.vector.tensor_tensor(out=outT[:, crs], in0=outT[:, crs],
                                            in1=u, op=OP.add)

            # ================= output transpose + store =================
            osb = p_out.tile([128, 5 * 64], F32, name="osb")
            for tt in range(5):
                po = ps_ms.tile([128, 64], F32, name="po", tag="ms")
                nc.tensor.transpose(po, outT[:, tt * 128:(tt + 1) * 128],
                                    id_t[0:64, 0:64])
                nc.vector.tensor_copy(out=osb[:, tt * 64:(tt + 1) * 64], in_=po)
            nc.sync.dma_start(
                out=out[sb * SB_CR:(sb + 1) * SB_CR, :].rearrange(
                    "(a b) f -> b a f", b=128),
                in_=osb.rearrange("p (a f) -> p a f", f=64))


def _wfac(w, pw):
    if pw > 0:
        return np.power(w, pw, dtype=np.float32)
    return (1.0 / (np.power(w, abs(pw), dtype=np.float32) + 1e-10)).astype(np.float32)


def _lrelu(x):
    return np.where(x >= 0, x, 0.01 * x)


def _sample_gate_shifts(ew_full, fea0_full, params):
    """Exact forward on a sample of crystals to bound per-head gate ranges.

    Returns (G[9], Gc[3]): per-head shifts = sampled min(gate) - 30.
    """
    S = 2048
    idx = np.linspace(0, C - 1, S).astype(np.int64)
    nodes = (idx[:, None] * NPC + np.arange(NPC)[None, :]).ravel()
    fea = fea0_full[nodes].astype(np.float32)          # (S*5, 64)
    wgt = ew_full[nodes].astype(np.float32)            # (S*5, 1)
    G = np.zeros(NG * NH, np.float32)
    Gc = np.zeros(NH, np.float32)
    fs = fea.reshape(S, NPC, FEA)
    for l in range(NG):
        upd = np.zeros_like(fs)
        for h in range(NH):
            p = params["graphs"][l][h]
            W1g = np.asarray(p["gate"]["hidden"][0]["W"], np.float32)
            b1g = np.asarray(p["gate"]["hidden"][0]["b"], np.float32)
            W2g = np.asarray(p["gate"]["out"]["W"], np.float32)
            b2g = np.asarray(p["gate"]["out"]["b"], np.float32)
            W1m = np.asarray(p["msg"]["hidden"][0]["W"], np.float32)
            b1m = np.asarray(p["msg"]["hidden"][0]["b"], np.float32)
            W2m = np.asarray(p["msg"]["out"]["W"], np.float32)
            b2m = np.asarray(p["msg"]["out"]["b"], np.float32)
            pw = float(np.asarray(p["pow"])[0])
            cat = np.concatenate(
                [np.repeat(fs, NPC, axis=1),
                 np.tile(fs, (1, NPC, 1))], axis=2).reshape(S * 25, 2 * FEA)
            gate = (_lrelu(cat @ W1g + b1g) @ W2g).ravel()  # no b2g (cancels)
            G[l * NH + h] = gate.min()
            gate = gate.reshape(S, NPC, NPC)
            gmax = gate.max(axis=2, keepdims=True)
            wf = _wfac(wgt.reshape(S, NPC)[:, None, :], pw)
            e = wf * np.exp(gate - gmax)
            coef = e / (e.sum(axis=2, keepdims=True) + 1e-10)
            msg = (_lrelu(cat @ W1m + b1m) @ W2m + b2m).reshape(S, NPC, NPC, FEA)
            upd += (coef[..., None] * msg).sum(axis=2) / NH
        fs = fs + upd
    for h in range(NH):
        p = params["cry"][h]
        W1g = np.asarray(p["gate"]["hidden"][0]["W"], np.float32)
        b1g = np.asarray(p["gate"]["hidden"][0]["b"], np.float32)
        W2g = np.asarray(p["gate"]["out"]["W"], np.float32)
        x = fs.reshape(S * NPC, FEA)
        gate = (_lrelu(x @ W1g + b1g) @ W2g).ravel()
        Gc[h] = gate.min()
    return G - 30.0, Gc - 30.0  # shift; drain bias = -G


def _pack_inputs(elem_weights, elem_fea, params):
    ew_full = np.asarray(elem_weights, np.float32)            # (N,1)
    emb_W = np.asarray(params["embedding"]["W"], np.float32)  # (200,63)
    emb_b = np.asarray(params["embedding"]["b"], np.float32)
    ef_full = np.asarray(elem_fea, np.float32)

    # fea0 on host only for the gate-shift sample
    fea0 = ef_full @ emb_W + emb_b
    fea0 = np.concatenate([fea0, ew_full], axis=1)
    G, Gc = _sample_gate_shifts(ew_full, fea0, params)

    w1 = np.zeros((128, 36 * 128), np.float32)
    w2 = np.zeros((128, 36 * 65), np.float32)
    b1 = np.zeros((128, 36), np.float32)
    b2 = np.zeros((65, 12), np.float32)
    for l in range(NG):
        for h in range(NH):
            hl = l * NH + h
            p = params["graphs"][l][h]
            for net, key in ((0, "gate"), (1, "msg")):
                W1 = np.asarray(p[key]["hidden"][0]["W"], np.float32)
                B1 = np.asarray(p[key]["hidden"][0]["b"], np.float32)
                for blk in range(2):
                    i = (hl * 2 + net) * 2 + blk
                    w1[:, i * 128:(i + 1) * 128] = W1[:, blk * 128:(blk + 1) * 128]
                    b1[:, i] = B1[blk * 128:(blk + 1) * 128]
            W2g = np.asarray(p["gate"]["out"]["W"], np.float32)
            W2m = np.asarray(p["msg"]["out"]["W"], np.float32)
            B2m = np.asarray(p["msg"]["out"]["b"], np.float32)
            # blocks 0,1 = msg halves; 2,3 = gate halves
            for blk in range(2):
                a = np.zeros((128, 65), np.float32)
                a[:, 0:64] = W2m[blk * 128:(blk + 1) * 128, :] / NH
                w2[:, (hl * 4 + blk) * 65:(hl * 4 + blk + 1) * 65] = a
                g = np.zeros((128, 65), np.float32)
                g[:, 64] = W2g[blk * 128:(blk + 1) * 128, 0]
                w2[:, (hl * 4 + 2 + blk) * 65:(hl * 4 + 3 + blk) * 65] = g
            b2[0:64, hl] = B2m / NH
            b2[64, hl] = -G[hl]

    w1c = np.zeros((64, 12 * 128), np.float32)
    w2c = np.zeros((128, 12 * 65), np.float32)
    b1c = np.zeros((128, 12), np.float32)
    for h in range(NH):
        p = params["cry"][h]
        for net, key in ((0, "gate"), (1, "msg")):
            W1 = np.asarray(p[key]["hidden"][0]["W"], np.float32)
            B1 = np.asarray(p[key]["hidden"][0]["b"], np.float32)
            for blk in range(2):
                i = (h * 2 + net) * 2 + blk
                w1c[:, i * 128:(i + 1) * 128] = W1[:, blk * 128:(blk + 1) * 128]
                b1c[:, i] = B1[blk * 128:(blk + 1) * 128]
        W2g = np.asarray(p["gate"]["out"]["W"], np.float32)
        W2m = np.asarray(p["msg"]["out"]["W"], np.float32)
        B2m = np.asarray(p["msg"]["out"]["b"], np.float32)
        for blk in range(2):
            a = np.zeros((128, 65), np.float32)
            a[:, 0:64] = W2m[blk * 128:(blk + 1) * 128, :] / NH
            w2c[:, (h * 4 + blk) * 65:(h * 4 + blk + 1) * 65] = a
            g = np.zeros((128, 65), np.float32)
            g[:, 64] = W2g[blk * 128:(blk + 1) * 128, 0]
            w2c[:, (h * 4 + 2 + blk) * 65:(h * 4 + 3 + blk) * 65] = g
        b2[0:64, 9 + h] = B2m / NH
        b2[64, 9 + h] = -Gc[h]

    shared = {
        "w1": w1.astype(BF), "w2": w2, "b1": b1, "b2": b2,
        "w1c": w1c, "w2c": w2c, "b1c": b1c,
        "wemba": np.ascontiguousarray(emb_W[0:128, :]),
        "wembb": np.ascontiguousarray(emb_W[128:200, :]),
        "bemb": emb_b.reshape(63, 1).astype(np.float32),
        "ident": np.eye(128, dtype=np.float32),
    }

    # per-head per-node wfac, per core
    in_maps = []
    for k in range(NCORES):
        n0 = k * N_CORE
        ewk = ew_full[n0:n0 + N_CORE, 0]
        wfg_k = np.zeros((NG * NH, NSB, 128, NPP), np.float32)
        wfc_k = np.zeros((NH, NSB, 128, NPP), np.float32)
        for l in range(NG):
            for h in range(NH):
                pw = float(np.asarray(params["graphs"][l][h]["pow"])[0])
                wfg_k[l * NH + h] = _wfac(ewk, pw).reshape(NSB, 128, NPP)
        for h in range(NH):
            pw = float(np.asarray(params["cry"][h]["pow"])[0])
            wfc_k[h] = _wfac(ewk, pw).reshape(NSB, 128, NPP)
        m = dict(shared)
        m["ef"] = np.ascontiguousarray(ef_full[n0:n0 + N_CORE])
        m["ew"] = np.ascontiguousarray(ew_full[n0:n0 + N_CORE, 0]).reshape(1, N_CORE)
        m["wfg"] = wfg_k
        m["wfc"] = wfc_k
        in_maps.append(m)
    return in_maps


def _check_idx(self_fea_idx, nbr_fea_idx, cry_elem_idx):
    nodes = np.arange(C * NPC, dtype=np.int64).reshape(C, NPC)
    ok = (np.array_equal(np.asarray(self_fea_idx).ravel(),
                         np.repeat(nodes, NPC, axis=1).ravel())
          and np.array_equal(np.asarray(nbr_fea_idx).ravel(),
                             np.tile(nodes, (1, NPC)).ravel())
          and np.array_equal(np.asarray(cry_elem_idx).ravel(),
                             np.repeat(np.arange(C, dtype=np.int64), NPC)))
    if not ok:
        raise ValueError("index inputs do not match the expected crystal structure")


def kernel(elem_weights, elem_fea, self_fea_idx, nbr_fea_idx, cry_elem_idx, params):
    import os
    _check_idx(self_fea_idx, nbr_fea_idx, cry_elem_idx)
    key = "nc" + os.environ.get("KERNEL_REPS", "1")
    if key not in _CACHE:
        _CACHE[key] = _build()
    nc = _CACHE[key]
    in_maps = _pack_inputs(elem_weights, elem_fea, params)
    trace = os.environ.get("KERNEL_TRACE") == "1"
    res = run_bass_kernel_spmd(nc, in_maps, core_ids=list(range(NCORES)),
                               trace=trace)
    _CACHE["last"] = res
    return np.concatenate([r["out"] for r in res.results], axis=0)


# revision 16
# speedup vs baseline: 4.0253x; 1.1569x over previous
"""Trainium2 Bass kernel for nn_DescriptorNetwork (gnn_message_passing).

Strategy:
 - Shard by crystal: core k owns crystals [3200k, 3200(k+1)) -> nodes/edges
   contiguous. Params replicated. All gathers are crystal-local.
 - Feature-major on-chip layout: feaT [64, nodes]; per-superblock streaming
   (640 crystals = 3200 nodes = 16000 edges per SB, 5 SBs per core).
 - Hidden layers of gate/msg nets: bf16 matmuls on catT [128, edges].
   cat gather = AP tricks (each crystal fully connected, 5 nodes).
 - gate output row rides the msg matmul as a 65th lhsT column -> one
   [65, chunk] PSUM tile, one ACT/DVE drain pass.
 - Attention softmax on [128, 125] tiles (partition p owns 5 crystals),
   reached via SBUF->SBUF reshape DMA of the gate row.
 - exp stabilization: constant per-head shift G (baked into drain bias),
   valid because softmax is shift-invariant; G chosen from an exact
   host-side forward of a sample of crystals so exp stays in range.
 - coeff broadcast across 64 feature partitions via PE outer product
   (ones[1,64] x coeff-row), PSUM-accumulation-free.
 - segment sums (5 edges/node, 5 nodes/crystal) = strided tensor_reduce.
"""
import sys
sys.path.insert(0, "/opt/trn_rl_repo")

import numpy as np
import ml_dtypes

import concourse.bass as bass
import concourse.bacc as bacc
import concourse.tile as tile
import concourse.mybir as mybir
from concourse.bass_utils import run_bass_kernel_spmd

F32 = mybir.dt.float32
F32R = mybir.dt.float32r
BF16 = mybir.dt.bfloat16
AF = mybir.ActivationFunctionType
OP = mybir.AluOpType
BF = ml_dtypes.bfloat16

NCORES = 8
C = 25600
NPC = 5
FEA = 64
EMB = 200
HID = 256
NG = 3          # graph layers
NH = 3          # heads per layer / cry heads
C_CORE = C // NCORES            # 3200 crystals
N_CORE = C_CORE * NPC           # 16000 nodes
E_CORE = N_CORE * NPC           # 80000 edges
SB_CR = 640                     # crystals per superblock
NSB = C_CORE // SB_CR           # 5
N_SB = SB_CR * NPC              # 3200 nodes / SB
E_SB = N_SB * NPC               # 16000 edges / SB
ECH = 500                       # edge chunk (<=512 psum cols)
NECH = E_SB // ECH              # 32
CRCH = 400                      # cry node chunk
NCRCH = N_SB // CRCH            # 8
EPP = E_SB // 128               # 125 edges per partition (5 crystals)
NPP = N_SB // 128               # 25 nodes per partition
CPP = SB_CR // 128              # 5 crystals per partition
NGRP = 640                      # emb node group
EMSG_W = 1000                   # emsg window (edges)
CEMSG_W = 800                   # cry emsg window (nodes)

_CACHE = {}


def _build():
    nc = bacc.Bacc(None, target_bir_lowering=False)

    ef = nc.dram_tensor("ef", [N_CORE, EMB], F32, kind="ExternalInput")
    ew = nc.dram_tensor("ew", [1, N_CORE], F32R, kind="ExternalInput")
    wfg = nc.dram_tensor("wfg", [NG * NH, NSB, 128, NPP], F32, kind="ExternalInput")
    wfc = nc.dram_tensor("wfc", [NH, NSB, 128, NPP], F32, kind="ExternalInput")
    w1 = nc.dram_tensor("w1", [128, 36 * 128], BF16, kind="ExternalInput")
    w2 = nc.dram_tensor("w2", [128, 36 * 65], F32, kind="ExternalInput")
    b1 = nc.dram_tensor("b1", [128, 36], F32, kind="ExternalInput")
    b2 = nc.dram_tensor("b2", [65, 12], F32, kind="ExternalInput")
    w1c = nc.dram_tensor("w1c", [64, 12 * 128], F32, kind="ExternalInput")
    w2c = nc.dram_tensor("w2c", [128, 12 * 65], F32, kind="ExternalInput")
    b1c = nc.dram_tensor("b1c", [128, 12], F32, kind="ExternalInput")
    wemba = nc.dram_tensor("wemba", [128, 63], F32, kind="ExternalInput")
    wembb = nc.dram_tensor("wembb", [72, 63], F32, kind="ExternalInput")
    bemb = nc.dram_tensor("bemb", [63, 1], F32, kind="ExternalInput")
    ident = nc.dram_tensor("ident", [128, 128], F32, kind="ExternalInput")
    out = nc.dram_tensor("out", [C_CORE, FEA], F32, kind="ExternalOutput")
    import os
    if os.environ.get("KERNEL_DEBUG") == "1":
        dbg_fea = nc.dram_tensor("dbg_fea", [64, N_SB], F32, kind="ExternalOutput")
        dbg_mg = nc.dram_tensor("dbg_mg", [65, E_SB], F32, kind="ExternalOutput")
        dbg_coeff = nc.dram_tensor("dbg_coeff", [128, EPP], F32, kind="ExternalOutput")
        dbg_fea1 = nc.dram_tensor("dbg_fea1", [64, N_SB], F32, kind="ExternalOutput")
    else:
        dbg_fea = dbg_mg = dbg_coeff = dbg_fea1 = None

    with tile.TileContext(nc) as tc:
        _emit(nc, tc, locals())
    nc.finalize()
    return nc


def _emit(nc, tc, t):
    ef, ew, wfg, wfc = t["ef"], t["ew"], t["wfg"], t["wfc"]
    w1, w2, b1, b2 = t["w1"], t["w2"], t["b1"], t["b2"]
    w1c, w2c, b1c = t["w1c"], t["w2c"], t["b1c"]
    wemba, wembb, bemb, ident, out = t["wemba"], t["wembb"], t["bemb"], t["ident"], t["out"]
    dbg_fea, dbg_mg, dbg_coeff, dbg_fea1 = t["dbg_fea"], t["dbg_mg"], t["dbg_coeff"], t["dbg_fea1"]

    import contextlib
    ctx = contextlib.ExitStack()
    with ctx:
        const = ctx.enter_context(tc.tile_pool(name="const", bufs=1))
        p_fea = ctx.enter_context(tc.tile_pool(name="p_fea", bufs=1))
        p_cat = ctx.enter_context(tc.tile_pool(name="p_cat", bufs=1))
        p_mg = ctx.enter_context(tc.tile_pool(name="p_mg", bufs=1))
        p_h1 = ctx.enter_context(tc.tile_pool(name="p_h1", bufs=2))
        p_sm = ctx.enter_context(tc.tile_pool(name="p_sm", bufs=2))
        p_c4 = ctx.enter_context(tc.tile_pool(name="p_c4", bufs=1))
        p_em = ctx.enter_context(tc.tile_pool(name="p_em", bufs=2))
        p_ef = ctx.enter_context(tc.tile_pool(name="p_ef", bufs=3))
        p_eft = ctx.enter_context(tc.tile_pool(name="p_eft", bufs=1))
        p_out = ctx.enter_context(tc.tile_pool(name="p_out", bufs=1))
        p_wf = ctx.enter_context(tc.tile_pool(name="p_wf", bufs=3))

        ps_h1 = ctx.enter_context(tc.tile_pool(name="ps_h1", bufs=1, space="PSUM"))
        ps_mg = ctx.enter_context(tc.tile_pool(name="ps_mg", bufs=2, space="PSUM"))
        ps_ms = ctx.enter_context(tc.tile_pool(name="ps_ms", bufs=2, space="PSUM"))

        # ---- resident params ----
        w1_t = const.tile([128, 36 * 128], BF16, name="w1t")
        nc.sync.dma_start(out=w1_t, in_=w1[:, :])
        b1_t = const.tile([128, 36], F32, name="b1t")
        nc.sync.dma_start(out=b1_t, in_=b1[:, :])
        b2_t = const.tile([65, 12], F32, name="b2t")
        nc.sync.dma_start(out=b2_t, in_=b2[:, :])
        b1c_t = const.tile([128, 12], F32, name="b1ct")
        nc.sync.dma_start(out=b1c_t, in_=b1c[:, :])
        bemb_t = const.tile([63, 1], F32, name="bembt")
        nc.sync.dma_start(out=bemb_t, in_=bemb[:, :])
        id_t = const.tile([128, 128], F32, name="idt")
        nc.sync.dma_start(out=id_t, in_=ident[:, :])

        # f32r params go through a DVE copy (producer must round to f32r)
        w2_t = const.tile([128, 36 * 65], F32R, name="w2t")
        w1c_t = const.tile([64, 12 * 128], F32R, name="w1ct")
        w2c_t = const.tile([128, 12 * 65], F32R, name="w2ct")
        wea_t = const.tile([128, 63], F32R, name="weat")
        web_t = const.tile([72, 63], F32R, name="webt")
        with tc.tile_pool(name="stg", bufs=1) as stg:
            for (dst, srcd, shp) in ((w2_t, w2, [128, 36 * 65]),
                                     (w1c_t, w1c, [64, 12 * 128]),
                                     (w2c_t, w2c, [128, 12 * 65]),
                                     (wea_t, wemba, [128, 63]),
                                     (web_t, wembb, [72, 63])):
                for c0 in range(0, shp[1], 640):
                    c1 = min(c0 + 640, shp[1])
                    s = stg.tile([128, 640], F32, name="stg_s", tag="s")
                    nc.sync.dma_start(out=s[0:shp[0], 0:c1 - c0],
                                      in_=srcd[:, c0:c1])
                    nc.vector.tensor_copy(out=dst[:, c0:c1],
                                          in_=s[0:shp[0], 0:c1 - c0])
        ones_s = const.tile([128, 64], F32, name="oness")
        nc.vector.memset(ones_s, 1.0)
        ones_t = const.tile([128, 64], F32R, name="onest")
        nc.vector.tensor_copy(out=ones_t, in_=ones_s)

        def bcast_inner5(ap2d, n):
            """[P, n] -> [P, n, 5] with 0-step inner (broadcast each col 5x)."""
            return bass.AP(tensor=ap2d.tensor, offset=ap2d.offset,
                           ap=[ap2d.ap[0], [ap2d.ap[1][0], n], [0, 5]])

        import os as _os
        reps = int(_os.environ.get("KERNEL_REPS", "1"))
        for sb in [s for r in range(reps) for s in range(NSB)]:
            nbase = sb * N_SB

            # ================= embedding =================
            feaT = p_fea.tile([64, N_SB], F32R, name="feaT")
            for g in range(N_SB // NGRP):
                hi = p_eft.tile([128, NGRP], F32R, name="efhi")
                lo = p_eft.tile([72, NGRP], F32R, name="eflo")
                rows = nbase + g * NGRP
                eft = p_ef.tile([128, 5 * EMB], F32, name="eft")
                nc.sync.dma_start(
                    out=eft.rearrange("p (t f) -> p t f", t=5),
                    in_=ef[rows:rows + NGRP, :].rearrange("(t p) f -> p t f", p=128))
                for tt in range(NGRP // 128):
                    ph = ps_ms.tile([128, 128], F32, name="pst", tag="ms")
                    nc.tensor.transpose(ph, eft[:, tt * EMB:tt * EMB + 128], id_t)
                    nc.vector.tensor_copy(out=hi[:, tt * 128:(tt + 1) * 128], in_=ph)
                    pl = ps_ms.tile([72, 128], F32, name="psl", tag="ms")
                    nc.tensor.transpose(pl, eft[:, tt * EMB + 128:tt * EMB + 200], id_t)
                    nc.vector.tensor_copy(out=lo[:, tt * 128:(tt + 1) * 128], in_=pl)
                for half in range(2):
                    cols = slice(half * (NGRP // 2), (half + 1) * (NGRP // 2))
                    pe = ps_ms.tile([63, NGRP // 2], F32, name="pse", tag="ms")
                    nc.tensor.matmul(out=pe, lhsT=wea_t, rhs=hi[:, cols],
                                     start=True, stop=False)
                    nc.tensor.matmul(out=pe, lhsT=web_t, rhs=lo[:, cols],
                                     start=False, stop=True)
                    dst = slice(g * NGRP + half * (NGRP // 2),
                                g * NGRP + (half + 1) * (NGRP // 2))
                    nc.scalar.activation(out=feaT[0:63, dst], in_=pe,
                                         func=AF.Identity, bias=bemb_t, scale=1.0)
            nc.sync.dma_start(out=feaT[63:64, :],
                              in_=ew[0:1, nbase:nbase + N_SB])
            if sb == 0 and dbg_fea is not None:
                nc.sync.dma_start(out=dbg_fea[:, :], in_=feaT.bitcast(F32))

            # ================= graph layers =================
            for l in range(NG):
                catq = [p_cat.tile([128, E_SB // 4], BF16, name=f"catT{q}",
                                   tag=f"catT{q}") for q in range(4)]
                for c in range(NECH):
                    crys = slice((c * ECH) // 25 * 5, ((c + 1) * ECH) // 25 * 5)
                    # self: node = 5cr + i, repeated 5x ; nbr: tile of 5
                    base = feaT[:, crys]
                    self_ap = bass.AP(tensor=base.tensor, offset=base.offset,
                                      ap=[base.ap[0], [5, ECH // 25], [1, 5], [0, 5]])
                    nbr_ap = bass.AP(tensor=base.tensor, offset=base.offset,
                                     ap=[base.ap[0], [5, ECH // 25], [0, 5], [1, 5]])
                    oc = catq[c // 8][:, (c % 8) * ECH:(c % 8 + 1) * ECH]
                    o3 = oc.rearrange("p (a b c) -> p a b c", b=5, c=5)
                    nc.vector.tensor_copy(out=o3[0:64], in_=self_ap)
                    nc.vector.tensor_copy(out=o3[64:128], in_=nbr_ap)

                for h in range(NH):
                    hl = l * NH + h
                    mgq = [p_mg.tile([65, E_SB // 4], F32, name=f"mg{q}",
                                     tag=f"mg{q}") for q in range(4)]
                    for c in range(NECH):
                        hg = ps_h1.tile([128, 1024], F32, name="hg")
                        hm = ps_h1.tile([128, 1024], F32, name="hm")
                        crhs = catq[c // 8][:, (c % 8) * ECH:(c % 8 + 1) * ECH]
                        for blk in range(2):
                            psl = slice(blk * 512, blk * 512 + ECH)
                            wcol = ((hl * 2 + 0) * 2 + blk) * 128
                            nc.tensor.matmul(out=hg[:, psl],
                                             lhsT=w1_t[:, wcol:wcol + 128],
                                             rhs=crhs, start=True, stop=True)
                            wcol = ((hl * 2 + 1) * 2 + blk) * 128
                            nc.tensor.matmul(out=hm[:, psl],
                                             lhsT=w1_t[:, wcol:wcol + 128],
                                             rhs=crhs, start=True, stop=True)
                        hgs = p_h1.tile([128, 2 * ECH], F32R, name="hgs")
                        hms = p_h1.tile([128, 2 * ECH], F32R, name="hms")
                        for blk in range(2):
                            bsl = slice(blk * ECH, (blk + 1) * ECH)
                            psl = slice(blk * 512, blk * 512 + ECH)
                            bg = (hl * 2 + 0) * 2 + blk
                            bm = (hl * 2 + 1) * 2 + blk
                            nc.scalar.activation(out=hgs[:, bsl], in_=hg[:, psl],
                                                 func=AF.Lrelu,
                                                 bias=b1_t[:, bg:bg + 1],
                                                 scale=1.0, alpha=0.01)
                            nc.scalar.activation(out=hms[:, bsl], in_=hm[:, psl],
                                                 func=AF.Lrelu,
                                                 bias=b1_t[:, bm:bm + 1],
                                                 scale=1.0, alpha=0.01)
                        pm = ps_mg.tile([65, ECH], F32, name="pm")
                        wb = hl * 4 * 65
                        nc.tensor.matmul(out=pm, lhsT=w2_t[:, wb:wb + 65],
                                         rhs=hms[:, 0:ECH], start=True, stop=False)
                        nc.tensor.matmul(out=pm, lhsT=w2_t[:, wb + 65:wb + 130],
                                         rhs=hms[:, ECH:2 * ECH], start=False, stop=False)
                        nc.tensor.matmul(out=pm, lhsT=w2_t[:, wb + 130:wb + 195],
                                         rhs=hgs[:, 0:ECH], start=False, stop=False)
                        nc.tensor.matmul(out=pm, lhsT=w2_t[:, wb + 195:wb + 260],
                                         rhs=hgs[:, ECH:2 * ECH], start=False, stop=True)
                        nc.vector.tensor_scalar(
                            out=mgq[c // 8][:, (c % 8) * ECH:(c % 8 + 1) * ECH],
                            in0=pm, scalar1=b2_t[:, hl:hl + 1],
                            scalar2=None, op0=OP.add)

                    # ---- attention smalls on [128, 125] ----
                    gsb = p_sm.tile([128, EPP], F32, name="gsb")
                    for q in range(4):
                        nc.sync.dma_start(
                            out=gsb[32 * q:32 * (q + 1), :],
                            in_=mgq[q][64:65, :].rearrange("o (p j) -> o p j", p=32))
                    wf = p_wf.tile([128, NPP], F32, name="wf")
                    nc.sync.dma_start(out=wf, in_=wfg[hl, sb])
                    t2 = p_sm.tile([128, EPP], F32, name="t2")
                    nc.scalar.activation(out=t2, in_=gsb, func=AF.Exp, scale=1.0)
                    t3 = p_sm.tile([128, EPP], F32, name="t3")
                    wf_ap = bass.AP(tensor=wf.tensor, offset=wf.offset,
                                    ap=[wf.ap[0], [5 * wf.ap[1][0], CPP],
                                        [0, 5], [wf.ap[1][0], 5]])
                    nc.vector.tensor_tensor(
                        out=t3.rearrange("p (a b c) -> p a b c", b=5, c=5),
                        in0=t2.rearrange("p (a b c) -> p a b c", b=5, c=5),
                        in1=wf_ap, op=OP.mult)
                    den = p_sm.tile([128, NPP], F32, name="den")
                    nc.vector.tensor_reduce(
                        out=den, in_=t3.rearrange("p (a b) -> p a b", b=5),
                        axis=mybir.AxisListType.X, op=OP.add)
                    rden = p_sm.tile([128, NPP], F32, name="rden")
                    nc.vector.tensor_scalar(out=rden, in0=den, scalar1=1e-10,
                                            scalar2=None, op0=OP.add)
                    nc.vector.reciprocal(out=rden, in_=rden)
                    coeff = p_sm.tile([128, EPP], F32R, name="coeff")
                    nc.vector.tensor_tensor(
                        out=coeff.rearrange("p (a b) -> p a b", b=5),
                        in0=t3.rearrange("p (a b) -> p a b", b=5),
                        in1=bcast_inner5(rden, NPP), op=OP.mult)
                    if sb == 0 and l == 0 and h == 0 and dbg_mg is not None:
                        nc.sync.dma_start(out=dbg_mg[:, :], in_=mgT)
                        nc.sync.dma_start(out=dbg_coeff[:, :], in_=coeff.bitcast(F32))
                    c4 = p_c4.tile([97, 4000], F32R, name="c4", tag="c4")
                    for k in range(4):
                        nc.sync.dma_start(
                            out=c4[32 * k:32 * k + 1, :].rearrange(
                                "o (p j) -> o p j", p=32),
                            in_=coeff[32 * k:32 * (k + 1), :])

                    # ---- apply coeff, segment-sum, update fea ----
                    for w in range(E_SB // EMSG_W):
                        em = p_em.tile([64, EMSG_W], F32, name="em")
                        for cc in range(EMSG_W // ECH):
                            c = w * (EMSG_W // ECH) + cc
                            k = (c * ECH) // 4000
                            koff = (c * ECH) % 4000
                            cb = ps_ms.tile([64, ECH], F32, name="cb", tag="ms")
                            nc.tensor.matmul(
                                out=cb, lhsT=ones_t[32 * k:32 * k + 1, :],
                                rhs=c4[32 * k:32 * k + 1, koff:koff + ECH],
                                start=True, stop=True, tile_position=(32 * k, 0))
                            nc.vector.tensor_tensor(
                                out=em[:, cc * ECH:(cc + 1) * ECH],
                                in0=mgq[c // 8][0:64, (c % 8) * ECH:(c % 8 + 1) * ECH],
                                in1=cb, op=OP.mult)
                        nodes = slice(w * (EMSG_W // 5), (w + 1) * (EMSG_W // 5))
                        u = p_em.tile([64, EMSG_W // 5], F32, name="u")
                        nc.vector.tensor_reduce(
                            out=u, in_=em.rearrange("p (a b) -> p a b", b=5),
                            axis=mybir.AxisListType.X, op=OP.add)
                        nc.vector.tensor_tensor(out=feaT[:, nodes],
                                                in0=feaT[:, nodes], in1=u,
                                                op=OP.add)

                if sb == 0 and l == 0 and dbg_fea1 is not None:
                    nc.sync.dma_start(out=dbg_fea1[:, :], in_=feaT.bitcast(F32))

            # ================= crystal pooling =================
            outT = p_out.tile([64, SB_CR], F32, name="outT")
            nc.vector.memset(outT, 0.0)
            for h in range(NH):
                mgcq = [p_mg.tile([65, N_SB // 4], F32, name=f"mgc{q}",
                                  tag=f"mg{q}") for q in range(4)]
                for c in range(NCRCH):
                    rsl = slice(c * CRCH, (c + 1) * CRCH)
                    hg = ps_h1.tile([128, 1024], F32, name="hg")
                    hm = ps_h1.tile([128, 1024], F32, name="hm")
                    for blk in range(2):
                        psl = slice(blk * 512, blk * 512 + CRCH)
                        wcol = ((h * 2 + 0) * 2 + blk) * 128
                        nc.tensor.matmul(out=hg[:, psl],
                                         lhsT=w1c_t[:, wcol:wcol + 128],
                                         rhs=feaT[:, rsl], start=True, stop=True)
                        wcol = ((h * 2 + 1) * 2 + blk) * 128
                        nc.tensor.matmul(out=hm[:, psl],
                                         lhsT=w1c_t[:, wcol:wcol + 128],
                                         rhs=feaT[:, rsl], start=True, stop=True)
                    hgs = p_h1.tile([128, 2 * CRCH], F32R, name="hgs")
                    hms = p_h1.tile([128, 2 * CRCH], F32R, name="hms")
                    for blk in range(2):
                        bsl = slice(blk * CRCH, (blk + 1) * CRCH)
                        psl = slice(blk * 512, blk * 512 + CRCH)
                        bg = (h * 2 + 0) * 2 + blk
                        bm = (h * 2 + 1) * 2 + blk
                        nc.scalar.activation(out=hgs[:, bsl], in_=hg[:, psl],
                                             func=AF.Lrelu, bias=b1c_t[:, bg:bg + 1],
                                             scale=1.0, alpha=0.01)
                        nc.scalar.activation(out=hms[:, bsl], in_=hm[:, psl],
                                             func=AF.Lrelu, bias=b1c_t[:, bm:bm + 1],
                                             scale=1.0, alpha=0.01)
                    pm = ps_mg.tile([65, CRCH], F32, name="pm")
                    wb = h * 4 * 65
                    nc.tensor.matmul(out=pm, lhsT=w2c_t[:, wb:wb + 65],
                                     rhs=hms[:, 0:CRCH], start=True, stop=False)
                    nc.tensor.matmul(out=pm, lhsT=w2c_t[:, wb + 65:wb + 130],
                                     rhs=hms[:, CRCH:2 * CRCH], start=False, stop=False)
                    nc.tensor.matmul(out=pm, lhsT=w2c_t[:, wb + 130:wb + 195],
                                     rhs=hgs[:, 0:CRCH], start=False, stop=False)
                    nc.tensor.matmul(out=pm, lhsT=w2c_t[:, wb + 195:wb + 260],
                                     rhs=hgs[:, CRCH:2 * CRCH], start=False, stop=True)
                    nc.vector.tensor_scalar(
                        out=mgcq[c // 2][:, (c % 2) * CRCH:(c % 2 + 1) * CRCH],
                        in0=pm, scalar1=b2_t[:, 9 + h:10 + h],
                        scalar2=None, op0=OP.add)

                gsb = p_sm.tile([128, NPP], F32, name="gsbc")
                for q in range(4):
                    nc.sync.dma_start(
                        out=gsb[32 * q:32 * (q + 1), :],
                        in_=mgcq[q][64:65, :].rearrange("o (p j) -> o p j", p=32))
                wf = p_wf.tile([128, NPP], F32, name="wf")
                nc.sync.dma_start(out=wf, in_=wfc[h, sb])
                t2 = p_sm.tile([128, NPP], F32, name="t2c")
                nc.scalar.activation(out=t2, in_=gsb, func=AF.Exp, scale=1.0)
                t3 = p_sm.tile([128, NPP], F32, name="t3c")
                nc.vector.tensor_tensor(out=t3, in0=t2, in1=wf, op=OP.mult)
                den = p_sm.tile([128, CPP], F32, name="denc")
                nc.vector.tensor_reduce(
                    out=den, in_=t3.rearrange("p (a b) -> p a b", b=5),
                    axis=mybir.AxisListType.X, op=OP.add)
                rden = p_sm.tile([128, CPP], F32, name="rdenc")
                nc.vector.tensor_scalar(out=rden, in0=den, scalar1=1e-10,
                                        scalar2=None, op0=OP.add)
                nc.vector.reciprocal(out=rden, in_=rden)
                coeff = p_sm.tile([128, NPP], F32R, name="coeffc")
                nc.vector.tensor_tensor(
                    out=coeff.rearrange("p (a b) -> p a b", b=5),
                    in0=t3.rearrange("p (a b) -> p a b", b=5),
                    in1=bcast_inner5(rden, CPP), op=OP.mult)
                c4 = p_c4.tile([97, 800], F32R, name="c4c", tag="c4")
                for k in range(4):
                    nc.sync.dma_start(
                        out=c4[32 * k:32 * k + 1, :].rearrange(
                            "o (p j) -> o p j", p=32),
                        in_=coeff[32 * k:32 * (k + 1), :])
                for w in range(N_SB // CEMSG_W):
                    em = p_em.tile([64, CEMSG_W], F32, name="em")
                    for cc in range(CEMSG_W // CRCH):
                        c = w * (CEMSG_W // CRCH) + cc
                        k = (c * CRCH) // 800
                        koff = (c * CRCH) % 800
                        cb = ps_ms.tile([64, CRCH], F32, name="cb", tag="ms")
                        nc.tensor.matmul(
                            out=cb, lhsT=ones_t[32 * k:32 * k + 1, :],
                            rhs=c4[32 * k:32 * k + 1, koff:koff + CRCH],
                            start=True, stop=True, tile_position=(32 * k, 0))
                        nc.vector.tensor_tensor(
                            out=em[:, cc * CRCH:(cc + 1) * CRCH],
                            in0=mgcq[c // 2][0:64, (c % 2) * CRCH:(c % 2 + 1) * CRCH],
                            in1=cb, op=OP.mult)
                    crs = slice(w * (CEMSG_W // 5), (w + 1) * (CEMSG_W // 5))
                    u = p_em.tile([64, CEMSG_W // 5], F32, name="u")
                    nc.vector.tensor_reduce(
                        out=u, in_=em.rearrange("p (a b) -> p a b", b=5),
                        axis=mybir.AxisListType.X, op=OP.add)
                    nc.vector.tensor_tensor(out=outT[:, crs], in0=outT[:, crs],
                                            in1=u, op=OP.add)

            # ================= output transpose + store =================
            osb = p_out.tile([128, 5 * 64], F32, name="osb")
            for tt in range(5):
                po = ps_ms.tile([128, 64], F32, name="po", tag="ms")
                nc.tensor.transpose(po, outT[:, tt * 128:(tt + 1) * 128],
                                    id_t[0:64, 0:64])
                nc.vector.tensor_copy(out=osb[:, tt * 64:(tt + 1) * 64], in_=po)
            nc.sync.dma_start(
                out=out[sb * SB_CR:(sb + 1) * SB_CR, :].rearrange(
                    "(a b) f -> b a f", b=128),
                in_=osb.rearrange("p (a f) -> p a f", f=64))


def _wfac(w, pw):
    if pw > 0:
        return np.power(w, pw, dtype=np.float32)
    return (1.0 / (np.power(w, abs(pw), dtype=np.float32) + 1e-10)).astype(np.float32)


def _lrelu(x):
    return np.where(x >= 0, x, 0.01 * x)


def _sample_gate_shifts(ew_full, fea0_full, params):
    """Exact forward on a sample of crystals to bound per-head gate ranges.

    Returns (G[9], Gc[3]): per-head shifts = sampled min(gate) - 30.
    """
    S = 2048
    idx = np.linspace(0, C - 1, S).astype(np.int64)
    nodes = (idx[:, None] * NPC + np.arange(NPC)[None, :]).ravel()
    fea = fea0_full[nodes].astype(np.float32)          # (S*5, 64)
    wgt = ew_full[nodes].astype(np.float32)            # (S*5, 1)
    G = np.zeros(NG * NH, np.float32)
    Gc = np.zeros(NH, np.float32)
    fs = fea.reshape(S, NPC, FEA)
    for l in range(NG):
        upd = np.zeros_like(fs)
        for h in range(NH):
            p = params["graphs"][l][h]
            W1g = np.asarray(p["gate"]["hidden"][0]["W"], np.float32)
            b1g = np.asarray(p["gate"]["hidden"][0]["b"], np.float32)
            W2g = np.asarray(p["gate"]["out"]["W"], np.float32)
            b2g = np.asarray(p["gate"]["out"]["b"], np.float32)
            W1m = np.asarray(p["msg"]["hidden"][0]["W"], np.float32)
            b1m = np.asarray(p["msg"]["hidden"][0]["b"], np.float32)
            W2m = np.asarray(p["msg"]["out"]["W"], np.float32)
            b2m = np.asarray(p["msg"]["out"]["b"], np.float32)
            pw = float(np.asarray(p["pow"])[0])
            cat = np.concatenate(
                [np.repeat(fs, NPC, axis=1),
                 np.tile(fs, (1, NPC, 1))], axis=2).reshape(S * 25, 2 * FEA)
            gate = (_lrelu(cat @ W1g + b1g) @ W2g).ravel()  # no b2g (cancels)
            G[l * NH + h] = gate.min()
            gate = gate.reshape(S, NPC, NPC)
            gmax = gate.max(axis=2, keepdims=True)
            wf = _wfac(wgt.reshape(S, NPC)[:, None, :], pw)
            e = wf * np.exp(gate - gmax)
            coef = e / (e.sum(axis=2, keepdims=True) + 1e-10)
            msg = (_lrelu(cat @ W1m + b1m) @ W2m + b2m).reshape(S, NPC, NPC, FEA)
            upd += (coef[..., None] * msg).sum(axis=2) / NH
        fs = fs + upd
    for h in range(NH):
        p = params["cry"][h]
        W1g = np.asarray(p["gate"]["hidden"][0]["W"], np.float32)
        b1g = np.asarray(p["gate"]["hidden"][0]["b"], np.float32)
        W2g = np.asarray(p["gate"]["out"]["W"], np.float32)
        x = fs.reshape(S * NPC, FEA)
        gate = (_lrelu(x @ W1g + b1g) @ W2g).ravel()
        Gc[h] = gate.min()
    return G - 30.0, Gc - 30.0  # shift; drain bias = -G


def _pack_inputs(elem_weights, elem_fea, params):
    ew_full = np.asarray(elem_weights, np.float32)            # (N,1)
    emb_W = np.asarray(params["embedding"]["W"], np.float32)  # (200,63)
    emb_b = np.asarray(params["embedding"]["b"], np.float32)
    ef_full = np.asarray(elem_fea, np.float32)

    # fea0 on host only for the gate-shift sample
    fea0 = ef_full @ emb_W + emb_b
    fea0 = np.concatenate([fea0, ew_full], axis=1)
    G, Gc = _sample_gate_shifts(ew_full, fea0, params)

    w1 = np.zeros((128, 36 * 128), np.float32)
    w2 = np.zeros((128, 36 * 65), np.float32)
    b1 = np.zeros((128, 36), np.float32)
    b2 = np.zeros((65, 12), np.float32)
    for l in range(NG):
        for h in range(NH):
            hl = l * NH + h
            p = params["graphs"][l][h]
            for net, key in ((0, "gate"), (1, "msg")):
                W1 = np.asarray(p[key]["hidden"][0]["W"], np.float32)
                B1 = np.asarray(p[key]["hidden"][0]["b"], np.float32)
                for blk in range(2):
                    i = (hl * 2 + net) * 2 + blk
                    w1[:, i * 128:(i + 1) * 128] = W1[:, blk * 128:(blk + 1) * 128]
                    b1[:, i] = B1[blk * 128:(blk + 1) * 128]
            W2g = np.asarray(p["gate"]["out"]["W"], np.float32)
            W2m = np.asarray(p["msg"]["out"]["W"], np.float32)
            B2m = np.asarray(p["msg"]["out"]["b"], np.float32)
            # blocks 0,1 = msg halves; 2,3 = gate halves
            for blk in range(2):
                a = np.zeros((128, 65), np.float32)
                a[:, 0:64] = W2m[blk * 128:(blk + 1) * 128, :] / NH
                w2[:, (hl * 4 + blk) * 65:(hl * 4 + blk + 1) * 65] = a
                g = np.zeros((128, 65), np.float32)
                g[:, 64] = W2g[blk * 128:(blk + 1) * 128, 0]
                w2[:, (hl * 4 + 2 + blk) * 65:(hl * 4 + 3 + blk) * 65] = g
            b2[0:64, hl] = B2m / NH
            b2[64, hl] = -G[hl]

    w1c = np.zeros((64, 12 * 128), np.float32)
    w2c = np.zeros((128, 12 * 65), np.float32)
    b1c = np.zeros((128, 12), np.float32)
    for h in range(NH):
        p = params["cry"][h]
        for net, key in ((0, "gate"), (1, "msg")):
            W1 = np.asarray(p[key]["hidden"][0]["W"], np.float32)
            B1 = np.asarray(p[key]["hidden"][0]["b"], np.float32)
            for blk in range(2):
                i = (h * 2 + net) * 2 + blk
                w1c[:, i * 128:(i + 1) * 128] = W1[:, blk * 128:(blk + 1) * 128]
                b1c[:, i] = B1[blk * 128:(blk + 1) * 128]
        W2g = np.asarray(p["gate"]["out"]["W"], np.float32)
        W2m = np.asarray(p["msg"]["out"]["W"], np.float32)
        B2m = np.asarray(p["msg"]["out"]["b"], np.float32)
        for blk in range(2):
            a = np.zeros((128, 65), np.float32)
            a[:, 0:64] = W2m[blk * 128:(blk + 1) * 128, :] / NH
            w2c[:, (h * 4 + blk) * 65:(h * 4 + blk + 1) * 65] = a
            g = np.zeros((128, 65), np.float32)
            g[:, 64] = W2g[blk * 128:(blk + 1) * 128, 0]
            w2c[:, (h * 4 + 2 + blk) * 65:(h * 4 + 3 + blk) * 65] = g
        b2[0:64, 9 + h] = B2m / NH
        b2[64, 9 + h] = -Gc[h]

    shared = {
        "w1": w1.astype(BF), "w2": w2, "b1": b1, "b2": b2,
        "w1c": w1c, "w2c": w2c, "b1c": b1c,
        "wemba": np.ascontiguousarray(emb_W[0:128, :]),
        "wembb": np.ascontiguousarray(emb_W[128:200, :]),
        "bemb": emb_b.reshape(63, 1).astype(np.float32),
        "ident": np.eye(128, dtype=np.float32),
    }

    # per-head per-node wfac, per core
    in_maps = []
    for k in range(NCORES):
        n0 = k * N_CORE
        ewk = ew_full[n0:n0 + N_CORE, 0]
        wfg_k = np.zeros((NG * NH, NSB, 128, NPP), np.float32)
        wfc_k = np.zeros((NH, NSB, 128, NPP), np.float32)
        for l in range(NG):
            for h in range(NH):
                pw = float(np.asarray(params["graphs"][l][h]["pow"])[0])
                wfg_k[l * NH + h] = _wfac(ewk, pw).reshape(NSB, 128, NPP)
        for h in range(NH):
            pw = float(np.asarray(params["cry"][h]["pow"])[0])
            wfc_k[h] = _wfac(ewk, pw).reshape(NSB, 128, NPP)
        m = dict(shared)
        m["ef"] = np.ascontiguousarray(ef_full[n0:n0 + N_CORE])
        m["ew"] = np.ascontiguousarray(ew_full[n0:n0 + N_CORE, 0]).reshape(1, N_CORE)
        m["wfg"] = wfg_k
        m["wfc"] = wfc_k
        in_maps.append(m)
    return in_maps


def _check_idx(self_fea_idx, nbr_fea_idx, cry_elem_idx):
    nodes = np.arange(C * NPC, dtype=np.int64).reshape(C, NPC)
    ok = (np.array_equal(np.asarray(self_fea_idx).ravel(),
                         np.repeat(nodes, NPC, axis=1).ravel())
          and np.array_equal(np.asarray(nbr_fea_idx).ravel(),
                             np.tile(nodes, (1, NPC)).ravel())
          and np.array_equal(np.asarray(cry_elem_idx).ravel(),
                             np.repeat(np.arange(C, dtype=np.int64), NPC)))
    if not ok:
        raise ValueError("index inputs do not match the expected crystal structure")


def kernel(elem_weights, elem_fea, self_fea_idx, nbr_fea_idx, cry_elem_idx, params):
    import os
    _check_idx(self_fea_idx, nbr_fea_idx, cry_elem_idx)
    key = "nc" + os.environ.get("KERNEL_REPS", "1")
    if key not in _CACHE:
        _CACHE[key] = _build()
    nc = _CACHE[key]
    in_maps = _pack_inputs(elem_weights, elem_fea, params)
    trace = os.environ.get("KERNEL_TRACE") == "1"
    res = run_bass_kernel_spmd(nc, in_maps, core_ids=list(range(NCORES)),
                               trace=trace)
    _CACHE["last"] = res
    return np.concatenate([r["out"] for r in res.results], axis=0)
